# revision 39
# baseline (speedup 1.0000x reference)
"""Sparse (half-causal) multi-head attention on 8 Trainium2 NeuronCores.

Problem: x[2,2048,1024] -> QKV proj (16 heads, dk=dv=64) -> scores with
half-causal mask (rows <1024 attend cols <1024 dense; rows >=1024 causal)
-> softmax -> out proj.

Sharding: 8 cores = 2 batches x 4 head-groups (4 heads each).  Each core
computes its batch's full QKV for its 4 heads (column-sharded W), attention
for those heads, and a partial output projection (row-sharded Wo).  Host
sums the 4 partials per batch.

v3 design:
 - QKV projections run as fp8e4 DoubleRow matmuls with residual
   compensation: W ~ (A + Ar)/32 and x ~ X + Xr, each term e4m3, giving
   bf16-level accuracy at 2x the f32r matmul rate (12 DR matmuls replace
   16 f32r-equivalent rows per piece); the 1/32 descale + bias fold into
   the psum->SBUF tensor_scalar
 - q-outer attention: per 512-wide q seg, scores S^T[k,q] (bf16 operands)
   land in PSUM [128,2,512] tiles, exp'd into bf16 p tiles; diag chunks
   compute/exp only the valid [128*t_tri, 512) column range
 - PV in q-major layout: out[q,dv] = P^T-slices @ [V|1] with F=65 (bf16),
   accumulated per 128-q-tile in PSUM [128,4,65]; col 64 = softmax denom.
   PSUM start marks the whole 2KB bank pending-zero, so only the first
   write into the bank carries start=True; each t-group's first write then
   self-zeroes its region
 - normalization per-partition (q on partitions): DVE reciprocal +
   tensor_scalar_mul; O[q,dv] -> O^T via DMA XBAR transpose (2 heads
   staged side by side); out-proj runs from O^T in bf16
 - PE-filler queue interleaves QKV/out-proj pieces between attention pairs
   so the PE never waits on the (Act-bound) exp stream
"""

import sys

if "/opt/trn_rl_repo" not in sys.path:
    sys.path.insert(0, "/opt/trn_rl_repo")

import ml_dtypes
import numpy as np

import concourse.bass as bass  # noqa: F401 (import registers engines)
import concourse.mybir as mybir
import concourse.tile as tile
from concourse import bacc
from concourse.bass_utils import run_bass_kernel_spmd

f32 = mybir.dt.float32
bf16 = mybir.dt.bfloat16
fp8 = mybir.dt.float8e4
DR = mybir.MatmulPerfMode.DoubleRow
AF = mybir.ActivationFunctionType
OP = mybir.AluOpType

D = 1024  # d_model
N = 2048  # n_ctx
HG = 256  # head-group width per core (4 heads x 64)
WS = 32.0  # fp8 weight pre-scale

# PE rows of filler emitted per attention pair (matches the ~2.1us the Act
# engine spends on the pair's two exps, minus the pair's own PE work)
FILL_ROWS = 2600


def make_tri() -> np.ndarray:
    """tri[kk, t, q'] = 1.0 if 128*t + kk <= q' else 0 — staircase masks."""
    kk = np.arange(128)[:, None, None]
    t = np.arange(4)[None, :, None]
    qp = np.arange(512)[None, None, :]
    return (128 * t + kk <= qp).astype(ml_dtypes.bfloat16)


def build_nc():
    nc = bacc.Bacc("TRN2", target_bir_lowering=False, debug=False)

    x8d = nc.declare_dram_parameter("x8", [D, N], fp8, isOutput=False)
    x8rd = nc.declare_dram_parameter("x8r", [D, N], fp8, isOutput=False)
    w8d = {}
    for w in ("q", "k", "v"):
        w8d[w, 0] = nc.declare_dram_parameter(f"w8{w}", [128, 8, HG], fp8, isOutput=False)
        w8d[w, 1] = nc.declare_dram_parameter(f"w8r{w}", [128, 8, HG], fp8, isOutput=False)
    bqd = nc.declare_dram_parameter("bq", [HG], f32, isOutput=False)
    bkd = nc.declare_dram_parameter("bk", [HG], f32, isOutput=False)
    bk8d = nc.declare_dram_parameter("bk8", [HG], fp8, isOutput=False)
    bvd = nc.declare_dram_parameter("bv", [HG], bf16, isOutput=False)  # pre-scaled x32
    wo = nc.declare_dram_parameter("wo", [HG, D], bf16, isOutput=False)
    trid = nc.declare_dram_parameter("tri", [128, 4, 512], bf16, isOutput=False)
    y = nc.declare_dram_parameter("y", [N, D], bf16, isOutput=True)

    x8_r = x8d[:].rearrange("(c p) n -> p c n", p=128)
    x8r_r = x8rd[:].rearrange("(c p) n -> p c n", p=128)

    with tile.TileContext(nc) as tc:
        with (
            tc.tile_pool(name="persist", bufs=1) as P1,
            tc.tile_pool(name="xtp", bufs=3) as XTP,
            tc.tile_pool(name="pp", bufs=6) as PP,
            tc.tile_pool(name="stg", bufs=4) as STG,
            tc.tile_pool(name="rp", bufs=8) as RP,
            tc.tile_pool(name="yp", bufs=6) as YP,
            tc.tile_pool(name="ps_s", bufs=2, space="PSUM") as PSA,
            tc.tile_pool(name="ps_pv", bufs=2, space="PSUM") as PVQ,
            tc.tile_pool(name="ps_b", bufs=2, space="PSUM") as PSB,
        ):
            # ---------- persistent tiles ----------
            w_sb = {}
            for w in ("q", "k", "v"):
                for r in (0, 1):
                    w_sb[w, r] = P1.tile([128, 8, HG], fp8, tag=f"w{w}{r}", name=f"w{w}{r}")
            wo_sb = P1.tile([128, 2, D], bf16, tag="wo")
            bq_sb = P1.tile([128, 2], f32, tag="bq")
            bk_sb = P1.tile([128, 2], f32, tag="bk")
            bv_sb = P1.tile([1, HG], bf16, tag="bv")
            ones_sb = P1.tile([1, 128], bf16, tag="ones")
            tri_sb = P1.tile([128, 4, 512], bf16, tag="tri")

            q8 = P1.tile([128, 2, N], fp8, tag="q8")
            k8 = P1.tile([128, 2, 2, N], fp8, tag="k8")
            bk8 = P1.tile([1, HG], fp8, tag="bk8")  # e4m3(16*bk): DR adds it twice
            ones8 = P1.tile([1, 512], fp8, tag="ones8")
            v1 = P1.tile([128, 16, 4, 65], bf16, tag="v1")
            att = P1.tile([128, 2, N], bf16, tag="att")

            dummy = P1.tile([1, 512], bf16, tag="dummy")
            nc.gpsimd.memset(dummy[:], 1.0)
            wps = PSB.tile([128, 512], f32, tag="b", name="warm")
            for i in range(10):
                nc.tensor.matmul(
                    wps[:], dummy[0:1, 0:128], dummy[0:1, :],
                    start=(i == 0), stop=(i == 9),
                )

            # ---------- initial loads ----------
            nc.sync.dma_start(w_sb["q", 0][:], w8d["q", 0][:])

            def emit_xt_load(n4, eng=None):
                eng = eng or nc.sync
                ns = slice(512 * n4, 512 * n4 + 512)
                tiles = []
                for i, (nm, src) in enumerate(
                    (("xa", x8_r), ("xb", x8_r), ("xar", x8r_r), ("xbr", x8r_r))
                ):
                    lo = 0 if nm in ("xa", "xar") else 4
                    t = XTP.tile([128, 4, 512], fp8, tag=nm, name=f"{nm}{n4}")
                    eng.dma_start(t[:], src[:, lo : lo + 4, ns])
                    tiles.append(t)
                return tiles  # [xa, xb, xar, xbr]

            xts = {0: emit_xt_load(0, eng=nc.sync)}
            nc.sync.dma_start(w_sb["q", 1][:], w8d["q", 1][:])
            nc.sync.dma_start(w_sb["k", 0][:], w8d["k", 0][:])
            nc.sync.dma_start(w_sb["k", 1][:], w8d["k", 1][:])
            nc.sync.dma_start(w_sb["v", 0][:], w8d["v", 0][:])
            nc.sync.dma_start(w_sb["v", 1][:], w8d["v", 1][:])
            nc.gpsimd.dma_start(bq_sb[:], bqd[:].rearrange("(m p) -> p m", p=128))
            nc.gpsimd.dma_start(bk_sb[:], bkd[:].rearrange("(m p) -> p m", p=128))
            nc.gpsimd.dma_start(bv_sb[:], bvd[None, :])
            nc.gpsimd.dma_start(bk8[:], bk8d[None, :])
            nc.gpsimd.memset(ones_sb[:], 1.0)
            nc.gpsimd.memset(ones8[:], 1.0)
            for h in range(4):
                nc.gpsimd.memset(v1[:, :, h, 64:65], 1.0)

            # ---------- QKV / out-proj piece emitters ----------
            def emit_qk_piece(which, m, n4):
                xa, xb, xar, xbr = xts[n4]
                ns = slice(512 * n4, 512 * n4 + 512)
                msl = slice(128 * m, 128 * m + 128)
                ps = PSB.tile([128, 512], f32, tag="b", name=f"{which}ps{m}{n4}")
                terms = ((0, (xa, xb)), (0, (xar, xbr)), (1, (xa, xb)))
                for ti, (wt, xp) in enumerate(terms):
                    wsb = w_sb[which, wt]
                    for hb in (0, 1):
                        for j in (0, 1):
                            c2 = 2 * (2 * hb + j)
                            nc.tensor.matmul(
                                ps[:],
                                wsb[:, c2 : c2 + 2, msl],
                                xp[hb][:, 2 * j : 2 * j + 2, :],
                                start=(ti == 0 and hb == 0 and j == 0),
                                stop=(which == "q" and ti == 2 and hb == 1 and j == 1),
                                perf_mode=DR,
                            )
                if which == "k":
                    # bias into psum (16*bk added via both DR subtiles), so the
                    # residual subtile compensates the k-side fp8 quantization
                    nc.tensor.matmul(
                        ps[:],
                        bk8[0:1, msl].unsqueeze(1).broadcast_to([1, 2, 128]),
                        ones8[0:1, :].unsqueeze(1).broadcast_to([1, 2, 512]),
                        start=False,
                        stop=True,
                        perf_mode=DR,
                    )
                    nc.vector.tensor_scalar_mul(k8[:, m, 0, ns], ps[:], 1.0 / WS)
                    nc.vector.scalar_tensor_tensor(
                        k8[:, m, 1, ns], ps[:], 1.0 / WS, k8[:, m, 0, ns],
                        OP.mult, OP.subtract,
                    )
                else:
                    nc.vector.tensor_scalar(
                        q8[:, m, ns], ps[:], 1.0 / WS, bq_sb[:, m : m + 1], OP.mult, OP.add
                    )

            def emit_v_piece(sch):
                xa, xb, xar, xbr = xts[sch // 4]
                so = 128 * (sch % 4)
                ps = PSB.tile([128, 256], f32, tag="b", name=f"vps{sch}")
                terms = ((0, (xa, xb)), (0, (xar, xbr)), (1, (xa, xb)))
                for ti, (wt, xp) in enumerate(terms):
                    wsb = w_sb["v", wt]
                    for hb in (0, 1):
                        for j in (0, 1):
                            c2 = 2 * (2 * hb + j)
                            nc.tensor.matmul(
                                ps[:],
                                xp[hb][:, 2 * j : 2 * j + 2, so : so + 128],
                                wsb[:, c2 : c2 + 2, :],
                                start=(ti == 0 and hb == 0 and j == 0),
                                stop=False,
                                perf_mode=DR,
                            )
                nc.tensor.matmul(ps[:], ones_sb[:], bv_sb[:], start=False, stop=True)
                nc.vector.tensor_scalar(
                    v1[:, sch, :, 0:64],
                    ps[:].rearrange("p (h d) -> p h d", h=4),
                    1.0 / WS,
                    None,
                    OP.mult,
                )

            yts = {}

            def emit_outproj_piece(T, dseg, tail=False):
                if dseg == 0:
                    yts[T] = YP.tile([128, D], bf16, tag="y", name=f"yt{T}")
                yt = yts[T]
                ps = PSB.tile([128, 512], f32, tag="b", name=f"yps{T}{dseg}")
                for hp in range(2):
                    nc.tensor.matmul(
                        ps[:],
                        att[:, hp, 128 * T : 128 * T + 128],
                        wo_sb[:, hp, 512 * dseg : 512 * dseg + 512],
                        start=(hp == 0),
                        stop=(hp == 1),
                    )
                dst = yt[:, 512 * dseg : 512 * dseg + 512]
                if tail and dseg == 0:
                    nc.scalar.copy(out=dst, in_=ps[:])
                else:
                    nc.vector.tensor_copy(out=dst, in_=ps[:])
                if tail:
                    nc.sync.dma_start(
                        y[128 * T : 128 * T + 128, 512 * dseg : 512 * dseg + 512], dst
                    )
                elif dseg == 1:
                    nc.sync.dma_start(y[128 * T : 128 * T + 128, :], yt[:])

            # ---------- PE filler queue ----------
            pending = []  # entries (key, rows, fn); key ('q'/'k', n4, m) / ('v', sch) / ('op', T)

            def push(key, rows, fn):
                pending.append((key, rows, fn))

            def require(pred):
                rest = []
                for e in pending:
                    if pred(e[0]):
                        e[2]()
                    else:
                        rest.append(e)
                pending[:] = rest

            def fill(budget):
                while pending and budget > 0:
                    key, rows, fn = pending.pop(0)
                    fn()
                    budget -= rows

            def push_qkv(n4, m_list=(0, 1)):
                for m in m_list:
                    push(("k", n4, m), 3072, lambda m=m: emit_qk_piece("k", m, n4))
                for sch in range(4 * n4, 4 * n4 + 4):
                    push(("v", sch, 0), 1792, lambda sch=sch: emit_v_piece(sch))
                for m in m_list:
                    push(("q", n4, m), 3072, lambda m=m: emit_qk_piece("q", m, n4))

            # ---------- attention substream ----------
            def emit_substream(half, s, hp, eager_tail=False):
                q0 = 1024 * half + 512 * s
                n4q = q0 // 512
                n_kc = 8 if half == 0 else 12 + 4 * s
                diag0 = 8 + 4 * s  # first diag kc (half 1 only)
                kc_max = [7 if half == 0 else 8 + 4 * s + t for t in range(4)]

                def off(kc):
                    return 128 * (kc - diag0) if (half == 1 and kc >= diag0) else 0

                require(lambda k: k[0] == "q" and k[1] == n4q and k[2] == hp)
                pvq = {
                    par: PVQ.tile(
                        [128, 4, 65], f32, tag="pvq", name=f"pvq{half}{s}{hp}{par}"
                    )
                    for par in (0, 1)
                }
                stage = STG.tile([128, 4, 128], bf16, tag="stg", name=f"stg{half}{s}{hp}")
                for p in range(n_kc // 2):
                    require(
                        lambda k, p=p: (
                            k[0] == "k" and k[1] <= (2 * p + 1) // 4 and k[2] == hp
                        )
                        or (k[0] == "v" and k[1] <= 2 * p + 1)
                    )
                    s_ps, pts = {}, {}
                    # score matmuls outrank recently-queued filler pieces on
                    # the PE heap: they are tiny and unblock the Act engine
                    with tc.high_priority(offset=400):
                        for par in (0, 1):
                            base = 64 * par
                            st = PSA.tile(
                                [128, 2, 512], f32, tag="s", name=f"s{half}{s}{hp}{par}{p}"
                            )
                            for j, kc in enumerate((2 * p, 2 * p + 1)):
                                o = off(kc)
                                nc.tensor.matmul(
                                    st[:, j, o:512],
                                    k8[base : base + 64, hp, :, 128 * kc : 128 * kc + 128],
                                    q8[base : base + 64, hp, q0 + o : q0 + 512]
                                    .unsqueeze(1)
                                    .broadcast_to([64, 2, 512 - o]),
                                    start=True,
                                    stop=True,
                                    perf_mode=DR,
                                )
                            s_ps[par] = st
                    for par in (0, 1):
                        pt = PP.tile(
                            [128, 2, 512], bf16, tag="p", name=f"p{half}{s}{hp}{par}{p}"
                        )
                        offs = [off(2 * p), off(2 * p + 1)]
                        if offs[1] == 0:
                            nc.scalar.activation(pt[:], s_ps[par][:], AF.Exp, scale=0.125)
                        else:
                            for j in (0, 1):
                                o = offs[j]
                                nc.scalar.activation(
                                    pt[:, j, o:512], s_ps[par][:, j, o:512], AF.Exp, scale=0.125
                                )
                        if half == 1:
                            for j, kc in enumerate((2 * p, 2 * p + 1)):
                                if kc >= diag0:
                                    tt = kc - diag0
                                    csl = slice(128 * tt, 128 * tt + 128)
                                    nc.vector.tensor_tensor(
                                        pt[:, j, csl], pt[:, j, csl], tri_sb[:, tt, csl], OP.mult
                                    )
                        pts[par] = pt
                    for par in (0, 1):
                        for t in range(4):
                            for j, kc in enumerate((2 * p, 2 * p + 1)):
                                if kc <= kc_max[t]:
                                    # start only on the bank's very first write:
                                    # it marks the whole 2KB bank pending-zero,
                                    # so each t-group's first write replaces
                                    # (self-zeroes) and later writes accumulate.
                                    nc.tensor.matmul(
                                        pvq[par][:, t, :],
                                        pts[par][:, j, 128 * t : 128 * t + 128],
                                        v1[:, kc, 2 * hp + par, :],
                                        start=(kc == 0 and t == 0),
                                        stop=(kc == kc_max[t]),
                                        skip_group_check=(not (kc == 0 and t == 0)),
                                    )
                    if eager_tail:
                        # finish q-tiles whose PV chains just stopped: norm,
                        # transpose, and out-proj inline so the kernel tail is
                        # a short per-tile pipeline instead of one serial drain
                        for t in range(4):
                            if kc_max[t] in (2 * p, 2 * p + 1):
                                for par in (0, 1):
                                    rc = RP.tile(
                                        [128, 1], f32, tag="rcp", name=f"rce{t}{par}"
                                    )
                                    nc.vector.reciprocal(rc[:], pvq[par][:, t, 64:65])
                                    nc.vector.tensor_scalar_mul(
                                        stage[:, t, 64 * par : 64 * par + 64],
                                        pvq[par][:, t, 0:64],
                                        rc[:, 0:1],
                                    )
                                nc.sync.dma_start(
                                    att[:, hp, q0 + 128 * t : q0 + 128 * t + 128],
                                    stage[:, t, :],
                                    transpose=True,
                                )
                                for dseg in range(2):
                                    emit_outproj_piece(8 * half + 4 * s + t, dseg)
                    fill(FILL_ROWS)
                if eager_tail:
                    return
                # normalize + transpose into att
                for par in (0, 1):
                    rcp = RP.tile([128, 4], f32, tag="rcp4", name=f"rcp{half}{s}{hp}{par}")
                    nc.vector.reciprocal(rcp[:], pvq[par][:, :, 64])
                    for t in range(4):
                        nc.vector.tensor_scalar_mul(
                            stage[:, t, 64 * par : 64 * par + 64],
                            pvq[par][:, t, 0:64],
                            rcp[:, t : t + 1],
                        )
                for t in range(4):
                    nc.sync.dma_start(
                        att[:, hp, q0 + 128 * t : q0 + 128 * t + 128],
                        stage[:, t, :],
                        transpose=True,
                    )

            # ---------- main emission ----------
            emit_qk_piece("q", 0, 0)
            emit_qk_piece("k", 0, 0)
            for sch in range(4):
                emit_v_piece(sch)
            xts[1] = emit_xt_load(1)
            nc.sync.dma_start(tri_sb[:], trid[:])
            nc.sync.dma_start(wo_sb[:], wo[:].rearrange("(c p) n -> p c n", p=128))
            push(("k", 0, 1), 3072, lambda: emit_qk_piece("k", 1, 0))
            push(("q", 0, 1), 3072, lambda: emit_qk_piece("q", 1, 0))
            push_qkv(1)

            SUBS = [(h, s, hp) for h, s in ((0, 0), (0, 1), (1, 0), (1, 1)) for hp in (0, 1)]
            for i, (half, s, hp) in enumerate(SUBS):
                if (half, s, hp) == (0, 0, 0):
                    xts[2] = emit_xt_load(2)
                    push_qkv(2)
                if (half, s, hp) == (0, 1, 0):
                    xts[3] = emit_xt_load(3)
                    push_qkv(3)
                # pull the next substream's qT piece ahead of this substream's
                # DVE backlog so its first scores aren't gated on the bias-add
                if i + 1 < len(SUBS):
                    nh, ns_, nhp = SUBS[i + 1]
                    nn4q = (1024 * nh + 512 * ns_) // 512
                    require(lambda k, n=nn4q, m=nhp: k[0] == "q" and k[1] == n and k[2] == m)
                emit_substream(half, s, hp, eager_tail=(i == len(SUBS) - 1))
                if hp == 1 and (half, s) != (1, 1):
                    for t in range(4):
                        T = 8 * half + 4 * s + t
                        for dseg in range(2):
                            push(
                                ("op", T, dseg),
                                1024,
                                lambda T=T, d=dseg: emit_outproj_piece(T, d),
                            )

            require(lambda k: True)

    nc.compile()
    return nc


_NC = None
_TRI = None


def _get_nc():
    global _NC, _TRI
    if _NC is None:
        _NC = build_nc()
        _TRI = make_tri()
    return _NC


def make_in_maps(x, Wq, bq, Wk, bk, Wv, bv, Wo):
    _get_nc()
    bf = ml_dtypes.bfloat16
    e4 = mybir.dt.np(fp8)
    x = np.asarray(x, np.float32)
    in_maps = []
    for core in range(8):
        b, g = core // 4, core % 4
        sl = slice(HG * g, HG * (g + 1))
        xt = np.ascontiguousarray(x[b].T)
        x8 = xt.astype(e4)
        x8r = (xt - x8.astype(np.float32)).astype(e4)
        m = {
            "x8": x8,
            "x8r": x8r,
            "bq": np.ascontiguousarray(np.asarray(bq, np.float32)[sl]),
            "bk": np.ascontiguousarray(np.asarray(bk, np.float32)[sl]),
            "bk8": np.ascontiguousarray(16.0 * np.asarray(bk, np.float32)[sl]).astype(e4),
            "bv": np.ascontiguousarray(WS * np.asarray(bv, np.float32)[sl]).astype(bf),
            "wo": np.ascontiguousarray(np.asarray(Wo, np.float32)[sl, :].astype(bf)),
            "tri": _TRI,
        }
        for nm, W in (("q", Wq), ("k", Wk), ("v", Wv)):
            ws = WS * np.asarray(W, np.float32)[:, sl]
            w8 = ws.astype(e4)
            w8r = (ws - w8.astype(np.float32)).astype(e4)
            # pre-rearranged (c p) m -> p c m so the DMA moves 2KB descriptors
            m[f"w8{nm}"] = np.ascontiguousarray(
                w8.reshape(8, 128, HG).transpose(1, 0, 2)
            )
            m[f"w8r{nm}"] = np.ascontiguousarray(
                w8r.reshape(8, 128, HG).transpose(1, 0, 2)
            )
        in_maps.append(m)
    return in_maps


def kernel(x, Wq, bq, Wk, bk, Wv, bv, Wo, _trace=False, _trace_kwargs=None):
    nc = _get_nc()
    in_maps = make_in_maps(x, Wq, bq, Wk, bk, Wv, bv, Wo)
    res = run_bass_kernel_spmd(
        nc, in_maps, list(range(8)), trace=_trace, **(_trace_kwargs or {})
    )
    out = np.zeros((2, N, D), np.float64)
    for core in range(8):
        out[core // 4] += np.asarray(res.results[core]["y"]).astype(np.float64)
    y = out.astype(np.float32)
    if _trace:
        return y, res
    return y


# revision 41
# speedup vs baseline: 1.0031x; 1.0031x over previous
"""Sparse (half-causal) multi-head attention on 8 Trainium2 NeuronCores.

Problem: x[2,2048,1024] -> QKV proj (16 heads, dk=dv=64) -> scores with
half-causal mask (rows <1024 attend cols <1024 dense; rows >=1024 causal)
-> softmax -> out proj.

Sharding: 8 cores = 2 batches x 4 head-groups (4 heads each).  Each core
computes its batch's full QKV for its 4 heads (column-sharded W), attention
for those heads, and a partial output projection (row-sharded Wo).  Host
sums the 4 partials per batch.

v3 design:
 - QKV projections run as fp8e4 DoubleRow matmuls with residual
   compensation: W ~ (A + Ar)/32 and x ~ X + Xr, each term e4m3, giving
   bf16-level accuracy at 2x the f32r matmul rate (12 DR matmuls replace
   16 f32r-equivalent rows per piece); the 1/32 descale + bias fold into
   the psum->SBUF tensor_scalar
 - q-outer attention: per 512-wide q seg, scores S^T[k,q] (bf16 operands)
   land in PSUM [128,2,512] tiles, exp'd into bf16 p tiles; diag chunks
   compute/exp only the valid [128*t_tri, 512) column range
 - PV in q-major layout: out[q,dv] = P^T-slices @ [V|1] with F=65 (bf16),
   accumulated per 128-q-tile in PSUM [128,4,65]; col 64 = softmax denom.
   PSUM start marks the whole 2KB bank pending-zero, so only the first
   write into the bank carries start=True; each t-group's first write then
   self-zeroes its region
 - normalization per-partition (q on partitions): DVE reciprocal +
   tensor_scalar_mul; O[q,dv] -> O^T via DMA XBAR transpose (2 heads
   staged side by side); out-proj runs from O^T in bf16
 - PE-filler queue interleaves QKV/out-proj pieces between attention pairs
   so the PE never waits on the (Act-bound) exp stream
"""

import sys

if "/opt/trn_rl_repo" not in sys.path:
    sys.path.insert(0, "/opt/trn_rl_repo")

import ml_dtypes
import numpy as np

import concourse.bass as bass  # noqa: F401 (import registers engines)
import concourse.mybir as mybir
import concourse.tile as tile
from concourse import bacc
from concourse.bass_utils import run_bass_kernel_spmd

f32 = mybir.dt.float32
bf16 = mybir.dt.bfloat16
fp8 = mybir.dt.float8e4
DR = mybir.MatmulPerfMode.DoubleRow
AF = mybir.ActivationFunctionType
OP = mybir.AluOpType

D = 1024  # d_model
N = 2048  # n_ctx
HG = 256  # head-group width per core (4 heads x 64)
WS = 32.0  # fp8 weight pre-scale

# PE rows of filler emitted per attention pair (matches the ~2.1us the Act
# engine spends on the pair's two exps, minus the pair's own PE work)
FILL_ROWS = 2600


def make_tri() -> np.ndarray:
    """tri[kk, t, q'] = 1.0 if 128*t + kk <= q' else 0 — staircase masks."""
    kk = np.arange(128)[:, None, None]
    t = np.arange(4)[None, :, None]
    qp = np.arange(512)[None, None, :]
    return (128 * t + kk <= qp).astype(ml_dtypes.bfloat16)


def build_nc():
    nc = bacc.Bacc("TRN2", target_bir_lowering=False, debug=False)

    x8d = nc.declare_dram_parameter("x8", [D, N], fp8, isOutput=False)
    x8rd = nc.declare_dram_parameter("x8r", [D, N], fp8, isOutput=False)
    w8d = {}
    for w in ("q", "k", "v"):
        w8d[w, 0] = nc.declare_dram_parameter(f"w8{w}", [128, 8, HG], fp8, isOutput=False)
        w8d[w, 1] = nc.declare_dram_parameter(f"w8r{w}", [128, 8, HG], fp8, isOutput=False)
    bqd = nc.declare_dram_parameter("bq", [HG], f32, isOutput=False)
    bkd = nc.declare_dram_parameter("bk", [HG], f32, isOutput=False)
    bk8d = nc.declare_dram_parameter("bk8", [HG], fp8, isOutput=False)
    bvd = nc.declare_dram_parameter("bv", [HG], bf16, isOutput=False)  # pre-scaled x32
    wo = nc.declare_dram_parameter("wo", [HG, D], bf16, isOutput=False)
    trid = nc.declare_dram_parameter("tri", [128, 4, 512], bf16, isOutput=False)
    y = nc.declare_dram_parameter("y", [N, D], bf16, isOutput=True)

    x8_r = x8d[:].rearrange("(c p) n -> p c n", p=128)
    x8r_r = x8rd[:].rearrange("(c p) n -> p c n", p=128)

    with tile.TileContext(nc) as tc:
        with (
            tc.tile_pool(name="persist", bufs=1) as P1,
            tc.tile_pool(name="xtp", bufs=3) as XTP,
            tc.tile_pool(name="pp", bufs=6) as PP,
            tc.tile_pool(name="stg", bufs=4) as STG,
            tc.tile_pool(name="rp", bufs=8) as RP,
            tc.tile_pool(name="yp", bufs=6) as YP,
            tc.tile_pool(name="ps_s", bufs=2, space="PSUM") as PSA,
            tc.tile_pool(name="ps_pv", bufs=2, space="PSUM") as PVQ,
            tc.tile_pool(name="ps_b", bufs=2, space="PSUM") as PSB,
        ):
            # ---------- persistent tiles ----------
            w_sb = {}
            for w in ("q", "k", "v"):
                for r in (0, 1):
                    w_sb[w, r] = P1.tile([128, 8, HG], fp8, tag=f"w{w}{r}", name=f"w{w}{r}")
            wo_sb = P1.tile([128, 2, D], bf16, tag="wo")
            bq_sb = P1.tile([128, 2], f32, tag="bq")
            bk_sb = P1.tile([128, 2], f32, tag="bk")
            bv_sb = P1.tile([1, HG], bf16, tag="bv")
            ones_sb = P1.tile([1, 128], bf16, tag="ones")
            tri_sb = P1.tile([128, 4, 512], bf16, tag="tri")

            q8 = P1.tile([128, 2, N], fp8, tag="q8")
            k8 = P1.tile([128, 2, 2, N], fp8, tag="k8")
            bk8 = P1.tile([1, HG], fp8, tag="bk8")  # e4m3(16*bk): DR adds it twice
            ones8 = P1.tile([1, 512], fp8, tag="ones8")
            v1 = P1.tile([128, 16, 4, 65], bf16, tag="v1")
            att = P1.tile([128, 2, N], bf16, tag="att")

            dummy = P1.tile([1, 512], bf16, tag="dummy")
            nc.gpsimd.memset(dummy[:], 1.0)
            wps = PSB.tile([128, 512], f32, tag="b", name="warm")
            for i in range(10):
                nc.tensor.matmul(
                    wps[:], dummy[0:1, 0:128], dummy[0:1, :],
                    start=(i == 0), stop=(i == 9),
                )

            # ---------- initial loads ----------
            nc.sync.dma_start(w_sb["q", 0][:], w8d["q", 0][:])

            def emit_xt_load(n4, eng=None):
                eng = eng or nc.sync
                ns = slice(512 * n4, 512 * n4 + 512)
                tiles = []
                for i, (nm, src) in enumerate(
                    (("xa", x8_r), ("xb", x8_r), ("xar", x8r_r), ("xbr", x8r_r))
                ):
                    lo = 0 if nm in ("xa", "xar") else 4
                    t = XTP.tile([128, 4, 512], fp8, tag=nm, name=f"{nm}{n4}")
                    eng.dma_start(t[:], src[:, lo : lo + 4, ns])
                    tiles.append(t)
                return tiles  # [xa, xb, xar, xbr]

            xts = {0: emit_xt_load(0, eng=nc.sync)}
            nc.sync.dma_start(w_sb["q", 1][:], w8d["q", 1][:])
            nc.sync.dma_start(w_sb["k", 0][:], w8d["k", 0][:])
            nc.sync.dma_start(w_sb["k", 1][:], w8d["k", 1][:])
            nc.sync.dma_start(w_sb["v", 0][:], w8d["v", 0][:])
            nc.sync.dma_start(w_sb["v", 1][:], w8d["v", 1][:])
            nc.gpsimd.dma_start(bq_sb[:], bqd[:].rearrange("(m p) -> p m", p=128))
            nc.gpsimd.dma_start(bk_sb[:], bkd[:].rearrange("(m p) -> p m", p=128))
            nc.gpsimd.dma_start(bv_sb[:], bvd[None, :])
            nc.gpsimd.dma_start(bk8[:], bk8d[None, :])
            nc.gpsimd.memset(ones_sb[:], 1.0)
            nc.gpsimd.memset(ones8[:], 1.0)
            for h in range(4):
                nc.gpsimd.memset(v1[:, :, h, 64:65], 1.0)

            # ---------- QKV / out-proj piece emitters ----------
            def emit_qk_piece(which, m, n4):
                xa, xb, xar, xbr = xts[n4]
                ns = slice(512 * n4, 512 * n4 + 512)
                msl = slice(128 * m, 128 * m + 128)
                ps = PSB.tile([128, 512], f32, tag="b", name=f"{which}ps{m}{n4}")
                terms = ((0, (xa, xb)), (0, (xar, xbr)), (1, (xa, xb)))
                for ti, (wt, xp) in enumerate(terms):
                    wsb = w_sb[which, wt]
                    for hb in (0, 1):
                        for j in (0, 1):
                            c2 = 2 * (2 * hb + j)
                            nc.tensor.matmul(
                                ps[:],
                                wsb[:, c2 : c2 + 2, msl],
                                xp[hb][:, 2 * j : 2 * j + 2, :],
                                start=(ti == 0 and hb == 0 and j == 0),
                                stop=(which == "q" and ti == 2 and hb == 1 and j == 1),
                                perf_mode=DR,
                            )
                if which == "k":
                    # bias into psum (16*bk added via both DR subtiles), so the
                    # residual subtile compensates the k-side fp8 quantization
                    nc.tensor.matmul(
                        ps[:],
                        bk8[0:1, msl].unsqueeze(1).broadcast_to([1, 2, 128]),
                        ones8[0:1, :].unsqueeze(1).broadcast_to([1, 2, 512]),
                        start=False,
                        stop=True,
                        perf_mode=DR,
                    )
                    nc.vector.tensor_scalar_mul(k8[:, m, 0, ns], ps[:], 1.0 / WS)
                    nc.vector.scalar_tensor_tensor(
                        k8[:, m, 1, ns], ps[:], 1.0 / WS, k8[:, m, 0, ns],
                        OP.mult, OP.subtract,
                    )
                else:
                    nc.vector.tensor_scalar(
                        q8[:, m, ns], ps[:], 1.0 / WS, bq_sb[:, m : m + 1], OP.mult, OP.add
                    )

            def emit_v_piece(sch):
                xa, xb, xar, xbr = xts[sch // 4]
                so = 128 * (sch % 4)
                ps = PSB.tile([128, 256], f32, tag="b", name=f"vps{sch}")
                terms = ((0, (xa, xb)), (0, (xar, xbr)), (1, (xa, xb)))
                for ti, (wt, xp) in enumerate(terms):
                    wsb = w_sb["v", wt]
                    for hb in (0, 1):
                        for j in (0, 1):
                            c2 = 2 * (2 * hb + j)
                            nc.tensor.matmul(
                                ps[:],
                                xp[hb][:, 2 * j : 2 * j + 2, so : so + 128],
                                wsb[:, c2 : c2 + 2, :],
                                start=(ti == 0 and hb == 0 and j == 0),
                                stop=False,
                                perf_mode=DR,
                            )
                nc.tensor.matmul(ps[:], ones_sb[:], bv_sb[:], start=False, stop=True)
                nc.vector.tensor_scalar(
                    v1[:, sch, :, 0:64],
                    ps[:].rearrange("p (h d) -> p h d", h=4),
                    1.0 / WS,
                    None,
                    OP.mult,
                )

            yts = {}

            def emit_outproj_piece(T, dseg, tail=False):
                if dseg == 0:
                    yts[T] = YP.tile([128, D], bf16, tag="y", name=f"yt{T}")
                yt = yts[T]
                ps = PSB.tile([128, 512], f32, tag="b", name=f"yps{T}{dseg}")
                for hp in range(2):
                    nc.tensor.matmul(
                        ps[:],
                        att[:, hp, 128 * T : 128 * T + 128],
                        wo_sb[:, hp, 512 * dseg : 512 * dseg + 512],
                        start=(hp == 0),
                        stop=(hp == 1),
                    )
                dst = yt[:, 512 * dseg : 512 * dseg + 512]
                if tail and dseg == 0:
                    nc.scalar.copy(out=dst, in_=ps[:])
                else:
                    nc.vector.tensor_copy(out=dst, in_=ps[:])
                if tail:
                    nc.sync.dma_start(
                        y[128 * T : 128 * T + 128, 512 * dseg : 512 * dseg + 512], dst
                    )
                elif dseg == 1:
                    nc.sync.dma_start(y[128 * T : 128 * T + 128, :], yt[:])

            # ---------- PE filler queue ----------
            pending = []  # entries (key, rows, fn); key ('q'/'k', n4, m) / ('v', sch) / ('op', T)

            def push(key, rows, fn):
                pending.append((key, rows, fn))

            def require(pred):
                rest = []
                for e in pending:
                    if pred(e[0]):
                        e[2]()
                    else:
                        rest.append(e)
                pending[:] = rest

            def fill(budget):
                while pending and budget > 0:
                    key, rows, fn = pending.pop(0)
                    fn()
                    budget -= rows

            def push_qkv(n4, m_list=(0, 1)):
                for m in m_list:
                    push(("k", n4, m), 3072, lambda m=m: emit_qk_piece("k", m, n4))
                for sch in range(4 * n4, 4 * n4 + 4):
                    push(("v", sch, 0), 1792, lambda sch=sch: emit_v_piece(sch))
                for m in m_list:
                    push(("q", n4, m), 3072, lambda m=m: emit_qk_piece("q", m, n4))

            # ---------- attention substream ----------
            def emit_substream(half, s, hp, eager_tail=False):
                q0 = 1024 * half + 512 * s
                n4q = q0 // 512
                n_kc = 8 if half == 0 else 12 + 4 * s
                diag0 = 8 + 4 * s  # first diag kc (half 1 only)
                kc_max = [7 if half == 0 else 8 + 4 * s + t for t in range(4)]

                def off(kc):
                    return 128 * (kc - diag0) if (half == 1 and kc >= diag0) else 0

                require(lambda k: k[0] == "q" and k[1] == n4q and k[2] == hp)
                pvq = {
                    par: PVQ.tile(
                        [128, 4, 65], f32, tag="pvq", name=f"pvq{half}{s}{hp}{par}"
                    )
                    for par in (0, 1)
                }
                stage = STG.tile([128, 4, 128], bf16, tag="stg", name=f"stg{half}{s}{hp}")
                for p in range(n_kc // 2):
                    require(
                        lambda k, p=p: (
                            k[0] == "k" and k[1] <= (2 * p + 1) // 4 and k[2] == hp
                        )
                        or (k[0] == "v" and k[1] <= 2 * p + 1)
                    )
                    s_ps, pts = {}, {}
                    # score matmuls outrank recently-queued filler pieces on
                    # the PE heap: they are tiny and unblock the Act engine
                    with tc.high_priority(offset=100):
                        for par in (0, 1):
                            base = 64 * par
                            st = PSA.tile(
                                [128, 2, 512], f32, tag="s", name=f"s{half}{s}{hp}{par}{p}"
                            )
                            for j, kc in enumerate((2 * p, 2 * p + 1)):
                                o = off(kc)
                                nc.tensor.matmul(
                                    st[:, j, o:512],
                                    k8[base : base + 64, hp, :, 128 * kc : 128 * kc + 128],
                                    q8[base : base + 64, hp, q0 + o : q0 + 512]
                                    .unsqueeze(1)
                                    .broadcast_to([64, 2, 512 - o]),
                                    start=True,
                                    stop=True,
                                    perf_mode=DR,
                                )
                            s_ps[par] = st
                    for par in (0, 1):
                        with tc.high_priority(offset=100):
                            pt = PP.tile(
                                [128, 2, 512], bf16, tag="p", name=f"p{half}{s}{hp}{par}{p}"
                            )
                            offs = [off(2 * p), off(2 * p + 1)]
                            if offs[1] == 0:
                                nc.scalar.activation(pt[:], s_ps[par][:], AF.Exp, scale=0.125)
                            else:
                                for j in (0, 1):
                                    o = offs[j]
                                    nc.scalar.activation(
                                        pt[:, j, o:512], s_ps[par][:, j, o:512], AF.Exp, scale=0.125
                                    )
                            if half == 1:
                                for j, kc in enumerate((2 * p, 2 * p + 1)):
                                    if kc >= diag0:
                                        tt = kc - diag0
                                        csl = slice(128 * tt, 128 * tt + 128)
                                        nc.vector.tensor_tensor(
                                            pt[:, j, csl], pt[:, j, csl], tri_sb[:, tt, csl], OP.mult
                                        )
                        pts[par] = pt
                    for par in (0, 1):
                        for t in range(4):
                            for j, kc in enumerate((2 * p, 2 * p + 1)):
                                if kc <= kc_max[t]:
                                    # start only on the bank's very first write:
                                    # it marks the whole 2KB bank pending-zero,
                                    # so each t-group's first write replaces
                                    # (self-zeroes) and later writes accumulate.
                                    nc.tensor.matmul(
                                        pvq[par][:, t, :],
                                        pts[par][:, j, 128 * t : 128 * t + 128],
                                        v1[:, kc, 2 * hp + par, :],
                                        start=(kc == 0 and t == 0),
                                        stop=(kc == kc_max[t]),
                                        skip_group_check=(not (kc == 0 and t == 0)),
                                    )
                    if eager_tail:
                        # finish q-tiles whose PV chains just stopped: norm,
                        # transpose, and out-proj inline so the kernel tail is
                        # a short per-tile pipeline instead of one serial drain
                        for t in range(4):
                            if kc_max[t] in (2 * p, 2 * p + 1):
                                for par in (0, 1):
                                    rc = RP.tile(
                                        [128, 1], f32, tag="rcp", name=f"rce{t}{par}"
                                    )
                                    nc.vector.reciprocal(rc[:], pvq[par][:, t, 64:65])
                                    nc.vector.tensor_scalar_mul(
                                        stage[:, t, 64 * par : 64 * par + 64],
                                        pvq[par][:, t, 0:64],
                                        rc[:, 0:1],
                                    )
                                nc.sync.dma_start(
                                    att[:, hp, q0 + 128 * t : q0 + 128 * t + 128],
                                    stage[:, t, :],
                                    transpose=True,
                                )
                                for dseg in range(2):
                                    emit_outproj_piece(8 * half + 4 * s + t, dseg)
                    fill(FILL_ROWS)
                if eager_tail:
                    return
                # normalize + transpose into att
                for par in (0, 1):
                    rcp = RP.tile([128, 4], f32, tag="rcp4", name=f"rcp{half}{s}{hp}{par}")
                    nc.vector.reciprocal(rcp[:], pvq[par][:, :, 64])
                    for t in range(4):
                        nc.vector.tensor_scalar_mul(
                            stage[:, t, 64 * par : 64 * par + 64],
                            pvq[par][:, t, 0:64],
                            rcp[:, t : t + 1],
                        )
                for t in range(4):
                    nc.sync.dma_start(
                        att[:, hp, q0 + 128 * t : q0 + 128 * t + 128],
                        stage[:, t, :],
                        transpose=True,
                    )

            # ---------- main emission ----------
            emit_qk_piece("q", 0, 0)
            emit_qk_piece("k", 0, 0)
            for sch in range(4):
                emit_v_piece(sch)
            xts[1] = emit_xt_load(1)
            nc.sync.dma_start(tri_sb[:], trid[:])
            nc.sync.dma_start(wo_sb[:], wo[:].rearrange("(c p) n -> p c n", p=128))
            push(("k", 0, 1), 3072, lambda: emit_qk_piece("k", 1, 0))
            push(("q", 0, 1), 3072, lambda: emit_qk_piece("q", 1, 0))
            push_qkv(1)

            SUBS = [(h, s, hp) for h, s in ((0, 0), (0, 1), (1, 0), (1, 1)) for hp in (0, 1)]
            for i, (half, s, hp) in enumerate(SUBS):
                if (half, s, hp) == (0, 0, 0):
                    xts[2] = emit_xt_load(2)
                    push_qkv(2)
                if (half, s, hp) == (0, 1, 0):
                    xts[3] = emit_xt_load(3)
                    push_qkv(3)
                # pull the next substream's qT piece ahead of this substream's
                # DVE backlog so its first scores aren't gated on the bias-add
                if i + 1 < len(SUBS):
                    nh, ns_, nhp = SUBS[i + 1]
                    nn4q = (1024 * nh + 512 * ns_) // 512
                    require(lambda k, n=nn4q, m=nhp: k[0] == "q" and k[1] == n and k[2] == m)
                emit_substream(half, s, hp, eager_tail=(i == len(SUBS) - 1))
                if hp == 1 and (half, s) != (1, 1):
                    for t in range(4):
                        T = 8 * half + 4 * s + t
                        for dseg in range(2):
                            push(
                                ("op", T, dseg),
                                1024,
                                lambda T=T, d=dseg: emit_outproj_piece(T, d),
                            )

            require(lambda k: True)

    nc.compile()
    return nc


_NC = None
_TRI = None


def _get_nc():
    global _NC, _TRI
    if _NC is None:
        _NC = build_nc()
        _TRI = make_tri()
    return _NC


def make_in_maps(x, Wq, bq, Wk, bk, Wv, bv, Wo):
    _get_nc()
    bf = ml_dtypes.bfloat16
    e4 = mybir.dt.np(fp8)
    x = np.asarray(x, np.float32)
    in_maps = []
    for core in range(8):
        b, g = core // 4, core % 4
        sl = slice(HG * g, HG * (g + 1))
        xt = np.ascontiguousarray(x[b].T)
        x8 = xt.astype(e4)
        x8r = (xt - x8.astype(np.float32)).astype(e4)
        m = {
            "x8": x8,
            "x8r": x8r,
            "bq": np.ascontiguousarray(np.asarray(bq, np.float32)[sl]),
            "bk": np.ascontiguousarray(np.asarray(bk, np.float32)[sl]),
            "bk8": np.ascontiguousarray(16.0 * np.asarray(bk, np.float32)[sl]).astype(e4),
            "bv": np.ascontiguousarray(WS * np.asarray(bv, np.float32)[sl]).astype(bf),
            "wo": np.ascontiguousarray(np.asarray(Wo, np.float32)[sl, :].astype(bf)),
            "tri": _TRI,
        }
        for nm, W in (("q", Wq), ("k", Wk), ("v", Wv)):
            ws = WS * np.asarray(W, np.float32)[:, sl]
            w8 = ws.astype(e4)
            w8r = (ws - w8.astype(np.float32)).astype(e4)
            # pre-rearranged (c p) m -> p c m so the DMA moves 2KB descriptors
            m[f"w8{nm}"] = np.ascontiguousarray(
                w8.reshape(8, 128, HG).transpose(1, 0, 2)
            )
            m[f"w8r{nm}"] = np.ascontiguousarray(
                w8r.reshape(8, 128, HG).transpose(1, 0, 2)
            )
        in_maps.append(m)
    return in_maps


def kernel(x, Wq, bq, Wk, bk, Wv, bv, Wo, _trace=False, _trace_kwargs=None):
    nc = _get_nc()
    in_maps = make_in_maps(x, Wq, bq, Wk, bk, Wv, bv, Wo)
    res = run_bass_kernel_spmd(
        nc, in_maps, list(range(8)), trace=_trace, **(_trace_kwargs or {})
    )
    out = np.zeros((2, N, D), np.float64)
    for core in range(8):
        out[core // 4] += np.asarray(res.results[core]["y"]).astype(np.float64)
    y = out.astype(np.float32)
    if _trace:
        return y, res
    return y


# revision 42
# speedup vs baseline: 1.0033x; 1.0002x over previous
"""Sparse (half-causal) multi-head attention on 8 Trainium2 NeuronCores.

Problem: x[2,2048,1024] -> QKV proj (16 heads, dk=dv=64) -> scores with
half-causal mask (rows <1024 attend cols <1024 dense; rows >=1024 causal)
-> softmax -> out proj.

Sharding: 8 cores = 2 batches x 4 head-groups (4 heads each).  Each core
computes its batch's full QKV for its 4 heads (column-sharded W), attention
for those heads, and a partial output projection (row-sharded Wo).  Host
sums the 4 partials per batch.

v3 design:
 - QKV projections run as fp8e4 DoubleRow matmuls with residual
   compensation: W ~ (A + Ar)/32 and x ~ X + Xr, each term e4m3, giving
   bf16-level accuracy at 2x the f32r matmul rate (12 DR matmuls replace
   16 f32r-equivalent rows per piece); the 1/32 descale + bias fold into
   the psum->SBUF tensor_scalar
 - q-outer attention: per 512-wide q seg, scores S^T[k,q] (bf16 operands)
   land in PSUM [128,2,512] tiles, exp'd into bf16 p tiles; diag chunks
   compute/exp only the valid [128*t_tri, 512) column range
 - PV in q-major layout: out[q,dv] = P^T-slices @ [V|1] with F=65 (bf16),
   accumulated per 128-q-tile in PSUM [128,4,65]; col 64 = softmax denom.
   PSUM start marks the whole 2KB bank pending-zero, so only the first
   write into the bank carries start=True; each t-group's first write then
   self-zeroes its region
 - normalization per-partition (q on partitions): DVE reciprocal +
   tensor_scalar_mul; O[q,dv] -> O^T via DMA XBAR transpose (2 heads
   staged side by side); out-proj runs from O^T in bf16
 - PE-filler queue interleaves QKV/out-proj pieces between attention pairs
   so the PE never waits on the (Act-bound) exp stream
"""

import sys

if "/opt/trn_rl_repo" not in sys.path:
    sys.path.insert(0, "/opt/trn_rl_repo")

import ml_dtypes
import numpy as np

import concourse.bass as bass  # noqa: F401 (import registers engines)
import concourse.mybir as mybir
import concourse.tile as tile
from concourse import bacc
from concourse.bass_utils import run_bass_kernel_spmd

f32 = mybir.dt.float32
bf16 = mybir.dt.bfloat16
fp8 = mybir.dt.float8e4
DR = mybir.MatmulPerfMode.DoubleRow
AF = mybir.ActivationFunctionType
OP = mybir.AluOpType

D = 1024  # d_model
N = 2048  # n_ctx
HG = 256  # head-group width per core (4 heads x 64)
WS = 32.0  # fp8 weight pre-scale

# PE rows of filler emitted per attention pair (matches the ~2.1us the Act
# engine spends on the pair's two exps, minus the pair's own PE work)
FILL_ROWS = 2600


def make_tri() -> np.ndarray:
    """tri[kk, t, q'] = 1.0 if 128*t + kk <= q' else 0 — staircase masks."""
    kk = np.arange(128)[:, None, None]
    t = np.arange(4)[None, :, None]
    qp = np.arange(512)[None, None, :]
    return (128 * t + kk <= qp).astype(ml_dtypes.bfloat16)


def build_nc():
    nc = bacc.Bacc("TRN2", target_bir_lowering=False, debug=False)

    x8d = nc.declare_dram_parameter("x8", [D, N], fp8, isOutput=False)
    x8rd = nc.declare_dram_parameter("x8r", [D, N], fp8, isOutput=False)
    w8d = {}
    for w in ("q", "k", "v"):
        w8d[w, 0] = nc.declare_dram_parameter(f"w8{w}", [128, 8, HG], fp8, isOutput=False)
        w8d[w, 1] = nc.declare_dram_parameter(f"w8r{w}", [128, 8, HG], fp8, isOutput=False)
    bqd = nc.declare_dram_parameter("bq", [HG], f32, isOutput=False)
    bkd = nc.declare_dram_parameter("bk", [HG], f32, isOutput=False)
    bk8d = nc.declare_dram_parameter("bk8", [HG], fp8, isOutput=False)
    bvd = nc.declare_dram_parameter("bv", [HG], bf16, isOutput=False)  # pre-scaled x32
    wo = nc.declare_dram_parameter("wo", [HG, D], bf16, isOutput=False)
    trid = nc.declare_dram_parameter("tri", [128, 4, 512], bf16, isOutput=False)
    y = nc.declare_dram_parameter("y", [N, D], bf16, isOutput=True)

    x8_r = x8d[:].rearrange("(c p) n -> p c n", p=128)
    x8r_r = x8rd[:].rearrange("(c p) n -> p c n", p=128)

    with tile.TileContext(nc) as tc:
        with (
            tc.tile_pool(name="persist", bufs=1) as P1,
            tc.tile_pool(name="xtp", bufs=3) as XTP,
            tc.tile_pool(name="pp", bufs=6) as PP,
            tc.tile_pool(name="stg", bufs=4) as STG,
            tc.tile_pool(name="rp", bufs=8) as RP,
            tc.tile_pool(name="yp", bufs=6) as YP,
            tc.tile_pool(name="ps_s", bufs=2, space="PSUM") as PSA,
            tc.tile_pool(name="ps_pv", bufs=2, space="PSUM") as PVQ,
            tc.tile_pool(name="ps_b", bufs=2, space="PSUM") as PSB,
        ):
            # ---------- persistent tiles ----------
            w_sb = {}
            for w in ("q", "k", "v"):
                for r in (0, 1):
                    w_sb[w, r] = P1.tile([128, 8, HG], fp8, tag=f"w{w}{r}", name=f"w{w}{r}")
            wo_sb = P1.tile([128, 2, D], bf16, tag="wo")
            bq_sb = P1.tile([128, 2], f32, tag="bq")
            bk_sb = P1.tile([128, 2], f32, tag="bk")
            bv_sb = P1.tile([1, HG], bf16, tag="bv")
            ones_sb = P1.tile([1, 128], bf16, tag="ones")
            tri_sb = P1.tile([128, 4, 512], bf16, tag="tri")

            q8 = P1.tile([128, 2, N], fp8, tag="q8")
            k8 = P1.tile([128, 2, 2, N], fp8, tag="k8")
            bk8 = P1.tile([1, HG], fp8, tag="bk8")  # e4m3(16*bk): DR adds it twice
            ones8 = P1.tile([1, 512], fp8, tag="ones8")
            v1 = P1.tile([128, 16, 4, 65], bf16, tag="v1")
            att = P1.tile([128, 2, N], bf16, tag="att")

            dummy = P1.tile([1, 512], bf16, tag="dummy")
            nc.gpsimd.memset(dummy[:], 1.0)
            wps = PSB.tile([128, 512], f32, tag="b", name="warm")
            for i in range(10):
                nc.tensor.matmul(
                    wps[:], dummy[0:1, 0:128], dummy[0:1, :],
                    start=(i == 0), stop=(i == 9),
                )

            # ---------- initial loads ----------
            nc.sync.dma_start(w_sb["q", 0][:], w8d["q", 0][:])

            def emit_xt_load(n4, eng=None):
                eng = eng or nc.sync
                ns = slice(512 * n4, 512 * n4 + 512)
                tiles = []
                for i, (nm, src) in enumerate(
                    (("xa", x8_r), ("xb", x8_r), ("xar", x8r_r), ("xbr", x8r_r))
                ):
                    lo = 0 if nm in ("xa", "xar") else 4
                    t = XTP.tile([128, 4, 512], fp8, tag=nm, name=f"{nm}{n4}")
                    eng.dma_start(t[:], src[:, lo : lo + 4, ns])
                    tiles.append(t)
                return tiles  # [xa, xb, xar, xbr]

            xts = {0: emit_xt_load(0, eng=nc.sync)}
            nc.sync.dma_start(w_sb["q", 1][:], w8d["q", 1][:])
            nc.sync.dma_start(w_sb["k", 0][:], w8d["k", 0][:])
            nc.sync.dma_start(w_sb["k", 1][:], w8d["k", 1][:])
            nc.sync.dma_start(w_sb["v", 0][:], w8d["v", 0][:])
            nc.sync.dma_start(w_sb["v", 1][:], w8d["v", 1][:])
            nc.gpsimd.dma_start(bq_sb[:], bqd[:].rearrange("(m p) -> p m", p=128))
            nc.gpsimd.dma_start(bk_sb[:], bkd[:].rearrange("(m p) -> p m", p=128))
            nc.gpsimd.dma_start(bv_sb[:], bvd[None, :])
            nc.gpsimd.dma_start(bk8[:], bk8d[None, :])
            nc.gpsimd.memset(ones_sb[:], 1.0)
            nc.gpsimd.memset(ones8[:], 1.0)
            for h in range(4):
                nc.gpsimd.memset(v1[:, :, h, 64:65], 1.0)

            # ---------- QKV / out-proj piece emitters ----------
            def emit_qk_piece(which, m, n4):
                xa, xb, xar, xbr = xts[n4]
                ns = slice(512 * n4, 512 * n4 + 512)
                msl = slice(128 * m, 128 * m + 128)
                ps = PSB.tile([128, 512], f32, tag="b", name=f"{which}ps{m}{n4}")
                terms = ((0, (xa, xb)), (0, (xar, xbr)), (1, (xa, xb)))
                for ti, (wt, xp) in enumerate(terms):
                    wsb = w_sb[which, wt]
                    for hb in (0, 1):
                        for j in (0, 1):
                            c2 = 2 * (2 * hb + j)
                            nc.tensor.matmul(
                                ps[:],
                                wsb[:, c2 : c2 + 2, msl],
                                xp[hb][:, 2 * j : 2 * j + 2, :],
                                start=(ti == 0 and hb == 0 and j == 0),
                                stop=(which == "q" and ti == 2 and hb == 1 and j == 1),
                                perf_mode=DR,
                            )
                if which == "k":
                    # bias into psum (16*bk added via both DR subtiles), so the
                    # residual subtile compensates the k-side fp8 quantization
                    nc.tensor.matmul(
                        ps[:],
                        bk8[0:1, msl].unsqueeze(1).broadcast_to([1, 2, 128]),
                        ones8[0:1, :].unsqueeze(1).broadcast_to([1, 2, 512]),
                        start=False,
                        stop=True,
                        perf_mode=DR,
                    )
                    # these gate the next substream's scores: outrank y-copies
                    # and V-copies on the DVE heap
                    with tc.high_priority(offset=100):
                        nc.vector.tensor_scalar_mul(k8[:, m, 0, ns], ps[:], 1.0 / WS)
                        nc.vector.scalar_tensor_tensor(
                            k8[:, m, 1, ns], ps[:], 1.0 / WS, k8[:, m, 0, ns],
                            OP.mult, OP.subtract,
                        )
                else:
                    with tc.high_priority(offset=100):
                        nc.vector.tensor_scalar(
                            q8[:, m, ns], ps[:], 1.0 / WS, bq_sb[:, m : m + 1],
                            OP.mult, OP.add,
                        )

            def emit_v_piece(sch):
                xa, xb, xar, xbr = xts[sch // 4]
                so = 128 * (sch % 4)
                ps = PSB.tile([128, 256], f32, tag="b", name=f"vps{sch}")
                terms = ((0, (xa, xb)), (0, (xar, xbr)), (1, (xa, xb)))
                for ti, (wt, xp) in enumerate(terms):
                    wsb = w_sb["v", wt]
                    for hb in (0, 1):
                        for j in (0, 1):
                            c2 = 2 * (2 * hb + j)
                            nc.tensor.matmul(
                                ps[:],
                                xp[hb][:, 2 * j : 2 * j + 2, so : so + 128],
                                wsb[:, c2 : c2 + 2, :],
                                start=(ti == 0 and hb == 0 and j == 0),
                                stop=False,
                                perf_mode=DR,
                            )
                nc.tensor.matmul(ps[:], ones_sb[:], bv_sb[:], start=False, stop=True)
                nc.vector.tensor_scalar(
                    v1[:, sch, :, 0:64],
                    ps[:].rearrange("p (h d) -> p h d", h=4),
                    1.0 / WS,
                    None,
                    OP.mult,
                )

            yts = {}

            def emit_outproj_piece(T, dseg, tail=False):
                if dseg == 0:
                    yts[T] = YP.tile([128, D], bf16, tag="y", name=f"yt{T}")
                yt = yts[T]
                ps = PSB.tile([128, 512], f32, tag="b", name=f"yps{T}{dseg}")
                for hp in range(2):
                    nc.tensor.matmul(
                        ps[:],
                        att[:, hp, 128 * T : 128 * T + 128],
                        wo_sb[:, hp, 512 * dseg : 512 * dseg + 512],
                        start=(hp == 0),
                        stop=(hp == 1),
                    )
                dst = yt[:, 512 * dseg : 512 * dseg + 512]
                if tail and dseg == 0:
                    nc.scalar.copy(out=dst, in_=ps[:])
                else:
                    nc.vector.tensor_copy(out=dst, in_=ps[:])
                if tail:
                    nc.sync.dma_start(
                        y[128 * T : 128 * T + 128, 512 * dseg : 512 * dseg + 512], dst
                    )
                elif dseg == 1:
                    nc.sync.dma_start(y[128 * T : 128 * T + 128, :], yt[:])

            # ---------- PE filler queue ----------
            pending = []  # entries (key, rows, fn); key ('q'/'k', n4, m) / ('v', sch) / ('op', T)

            def push(key, rows, fn):
                pending.append((key, rows, fn))

            def require(pred):
                rest = []
                for e in pending:
                    if pred(e[0]):
                        e[2]()
                    else:
                        rest.append(e)
                pending[:] = rest

            def fill(budget):
                while pending and budget > 0:
                    key, rows, fn = pending.pop(0)
                    fn()
                    budget -= rows

            def push_qkv(n4, m_list=(0, 1)):
                for m in m_list:
                    push(("k", n4, m), 3072, lambda m=m: emit_qk_piece("k", m, n4))
                for sch in range(4 * n4, 4 * n4 + 4):
                    push(("v", sch, 0), 1792, lambda sch=sch: emit_v_piece(sch))
                for m in m_list:
                    push(("q", n4, m), 3072, lambda m=m: emit_qk_piece("q", m, n4))

            # ---------- attention substream ----------
            def emit_substream(half, s, hp, eager_tail=False):
                q0 = 1024 * half + 512 * s
                n4q = q0 // 512
                n_kc = 8 if half == 0 else 12 + 4 * s
                diag0 = 8 + 4 * s  # first diag kc (half 1 only)
                kc_max = [7 if half == 0 else 8 + 4 * s + t for t in range(4)]

                def off(kc):
                    return 128 * (kc - diag0) if (half == 1 and kc >= diag0) else 0

                require(lambda k: k[0] == "q" and k[1] == n4q and k[2] == hp)
                pvq = {
                    par: PVQ.tile(
                        [128, 4, 65], f32, tag="pvq", name=f"pvq{half}{s}{hp}{par}"
                    )
                    for par in (0, 1)
                }
                stage = STG.tile([128, 4, 128], bf16, tag="stg", name=f"stg{half}{s}{hp}")
                for p in range(n_kc // 2):
                    require(
                        lambda k, p=p: (
                            k[0] == "k" and k[1] <= (2 * p + 1) // 4 and k[2] == hp
                        )
                        or (k[0] == "v" and k[1] <= 2 * p + 1)
                    )
                    s_ps, pts = {}, {}
                    # score matmuls outrank recently-queued filler pieces on
                    # the PE heap: they are tiny and unblock the Act engine
                    with tc.high_priority(offset=100):
                        for par in (0, 1):
                            base = 64 * par
                            st = PSA.tile(
                                [128, 2, 512], f32, tag="s", name=f"s{half}{s}{hp}{par}{p}"
                            )
                            for j, kc in enumerate((2 * p, 2 * p + 1)):
                                o = off(kc)
                                nc.tensor.matmul(
                                    st[:, j, o:512],
                                    k8[base : base + 64, hp, :, 128 * kc : 128 * kc + 128],
                                    q8[base : base + 64, hp, q0 + o : q0 + 512]
                                    .unsqueeze(1)
                                    .broadcast_to([64, 2, 512 - o]),
                                    start=True,
                                    stop=True,
                                    perf_mode=DR,
                                )
                            s_ps[par] = st
                    for par in (0, 1):
                        pt = PP.tile(
                            [128, 2, 512], bf16, tag="p", name=f"p{half}{s}{hp}{par}{p}"
                        )
                        offs = [off(2 * p), off(2 * p + 1)]
                        if offs[1] == 0:
                            nc.scalar.activation(pt[:], s_ps[par][:], AF.Exp, scale=0.125)
                        else:
                            for j in (0, 1):
                                o = offs[j]
                                nc.scalar.activation(
                                    pt[:, j, o:512], s_ps[par][:, j, o:512], AF.Exp, scale=0.125
                                )
                        if half == 1:
                            for j, kc in enumerate((2 * p, 2 * p + 1)):
                                if kc >= diag0:
                                    tt = kc - diag0
                                    csl = slice(128 * tt, 128 * tt + 128)
                                    nc.vector.tensor_tensor(
                                        pt[:, j, csl], pt[:, j, csl], tri_sb[:, tt, csl], OP.mult
                                    )
                        pts[par] = pt
                    for par in (0, 1):
                        for t in range(4):
                            for j, kc in enumerate((2 * p, 2 * p + 1)):
                                if kc <= kc_max[t]:
                                    # start only on the bank's very first write:
                                    # it marks the whole 2KB bank pending-zero,
                                    # so each t-group's first write replaces
                                    # (self-zeroes) and later writes accumulate.
                                    nc.tensor.matmul(
                                        pvq[par][:, t, :],
                                        pts[par][:, j, 128 * t : 128 * t + 128],
                                        v1[:, kc, 2 * hp + par, :],
                                        start=(kc == 0 and t == 0),
                                        stop=(kc == kc_max[t]),
                                        skip_group_check=(not (kc == 0 and t == 0)),
                                    )
                    if eager_tail:
                        # finish q-tiles whose PV chains just stopped: norm,
                        # transpose, and out-proj inline so the kernel tail is
                        # a short per-tile pipeline instead of one serial drain
                        for t in range(4):
                            if kc_max[t] in (2 * p, 2 * p + 1):
                                for par in (0, 1):
                                    rc = RP.tile(
                                        [128, 1], f32, tag="rcp", name=f"rce{t}{par}"
                                    )
                                    nc.vector.reciprocal(rc[:], pvq[par][:, t, 64:65])
                                    nc.vector.tensor_scalar_mul(
                                        stage[:, t, 64 * par : 64 * par + 64],
                                        pvq[par][:, t, 0:64],
                                        rc[:, 0:1],
                                    )
                                nc.sync.dma_start(
                                    att[:, hp, q0 + 128 * t : q0 + 128 * t + 128],
                                    stage[:, t, :],
                                    transpose=True,
                                )
                                for dseg in range(2):
                                    emit_outproj_piece(8 * half + 4 * s + t, dseg)
                    fill(FILL_ROWS)
                if eager_tail:
                    return
                # normalize + transpose into att
                for par in (0, 1):
                    rcp = RP.tile([128, 4], f32, tag="rcp4", name=f"rcp{half}{s}{hp}{par}")
                    nc.vector.reciprocal(rcp[:], pvq[par][:, :, 64])
                    for t in range(4):
                        nc.vector.tensor_scalar_mul(
                            stage[:, t, 64 * par : 64 * par + 64],
                            pvq[par][:, t, 0:64],
                            rcp[:, t : t + 1],
                        )
                for t in range(4):
                    nc.sync.dma_start(
                        att[:, hp, q0 + 128 * t : q0 + 128 * t + 128],
                        stage[:, t, :],
                        transpose=True,
                    )

            # ---------- main emission ----------
            emit_qk_piece("q", 0, 0)
            emit_qk_piece("k", 0, 0)
            for sch in range(4):
                emit_v_piece(sch)
            xts[1] = emit_xt_load(1)
            nc.sync.dma_start(tri_sb[:], trid[:])
            nc.sync.dma_start(wo_sb[:], wo[:].rearrange("(c p) n -> p c n", p=128))
            push(("k", 0, 1), 3072, lambda: emit_qk_piece("k", 1, 0))
            push(("q", 0, 1), 3072, lambda: emit_qk_piece("q", 1, 0))
            push_qkv(1)

            SUBS = [(h, s, hp) for h, s in ((0, 0), (0, 1), (1, 0), (1, 1)) for hp in (0, 1)]
            for i, (half, s, hp) in enumerate(SUBS):
                if (half, s, hp) == (0, 0, 0):
                    xts[2] = emit_xt_load(2)
                    push_qkv(2)
                if (half, s, hp) == (0, 1, 0):
                    xts[3] = emit_xt_load(3)
                    push_qkv(3)
                # pull the next substream's qT piece ahead of this substream's
                # DVE backlog so its first scores aren't gated on the bias-add
                if i + 1 < len(SUBS):
                    nh, ns_, nhp = SUBS[i + 1]
                    nn4q = (1024 * nh + 512 * ns_) // 512
                    require(lambda k, n=nn4q, m=nhp: k[0] == "q" and k[1] == n and k[2] == m)
                emit_substream(half, s, hp, eager_tail=(i == len(SUBS) - 1))
                if hp == 1 and (half, s) != (1, 1):
                    for t in range(4):
                        T = 8 * half + 4 * s + t
                        for dseg in range(2):
                            push(
                                ("op", T, dseg),
                                1024,
                                lambda T=T, d=dseg: emit_outproj_piece(T, d),
                            )

            require(lambda k: True)

    nc.compile()
    return nc


_NC = None
_TRI = None


def _get_nc():
    global _NC, _TRI
    if _NC is None:
        _NC = build_nc()
        _TRI = make_tri()
    return _NC


def make_in_maps(x, Wq, bq, Wk, bk, Wv, bv, Wo):
    _get_nc()
    bf = ml_dtypes.bfloat16
    e4 = mybir.dt.np(fp8)
    x = np.asarray(x, np.float32)
    in_maps = []
    for core in range(8):
        b, g = core // 4, core % 4
        sl = slice(HG * g, HG * (g + 1))
        xt = np.ascontiguousarray(x[b].T)
        x8 = xt.astype(e4)
        x8r = (xt - x8.astype(np.float32)).astype(e4)
        m = {
            "x8": x8,
            "x8r": x8r,
            "bq": np.ascontiguousarray(np.asarray(bq, np.float32)[sl]),
            "bk": np.ascontiguousarray(np.asarray(bk, np.float32)[sl]),
            "bk8": np.ascontiguousarray(16.0 * np.asarray(bk, np.float32)[sl]).astype(e4),
            "bv": np.ascontiguousarray(WS * np.asarray(bv, np.float32)[sl]).astype(bf),
            "wo": np.ascontiguousarray(np.asarray(Wo, np.float32)[sl, :].astype(bf)),
            "tri": _TRI,
        }
        for nm, W in (("q", Wq), ("k", Wk), ("v", Wv)):
            ws = WS * np.asarray(W, np.float32)[:, sl]
            w8 = ws.astype(e4)
            w8r = (ws - w8.astype(np.float32)).astype(e4)
            # pre-rearranged (c p) m -> p c m so the DMA moves 2KB descriptors
            m[f"w8{nm}"] = np.ascontiguousarray(
                w8.reshape(8, 128, HG).transpose(1, 0, 2)
            )
            m[f"w8r{nm}"] = np.ascontiguousarray(
                w8r.reshape(8, 128, HG).transpose(1, 0, 2)
            )
        in_maps.append(m)
    return in_maps


def kernel(x, Wq, bq, Wk, bk, Wv, bv, Wo, _trace=False, _trace_kwargs=None):
    nc = _get_nc()
    in_maps = make_in_maps(x, Wq, bq, Wk, bk, Wv, bv, Wo)
    res = run_bass_kernel_spmd(
        nc, in_maps, list(range(8)), trace=_trace, **(_trace_kwargs or {})
    )
    out = np.zeros((2, N, D), np.float64)
    for core in range(8):
        out[core // 4] += np.asarray(res.results[core]["y"]).astype(np.float64)
    y = out.astype(np.float32)
    if _trace:
        return y, res
    return y


# revision 43
# speedup vs baseline: 1.0203x; 1.0170x over previous
"""Sparse (half-causal) multi-head attention on 8 Trainium2 NeuronCores.

Problem: x[2,2048,1024] -> QKV proj (16 heads, dk=dv=64) -> scores with
half-causal mask (rows <1024 attend cols <1024 dense; rows >=1024 causal)
-> softmax -> out proj.

Sharding: 8 cores = 2 batches x 4 head-groups (4 heads each).  Each core
computes its batch's full QKV for its 4 heads (column-sharded W), attention
for those heads, and a partial output projection (row-sharded Wo).  Host
sums the 4 partials per batch.

v3 design:
 - QKV projections run as fp8e4 DoubleRow matmuls with residual
   compensation: W ~ (A + Ar)/32 and x ~ X + Xr, each term e4m3, giving
   bf16-level accuracy at 2x the f32r matmul rate (12 DR matmuls replace
   16 f32r-equivalent rows per piece); the 1/32 descale + bias fold into
   the psum->SBUF tensor_scalar
 - q-outer attention: per 512-wide q seg, scores S^T[k,q] (bf16 operands)
   land in PSUM [128,2,512] tiles, exp'd into bf16 p tiles; diag chunks
   compute/exp only the valid [128*t_tri, 512) column range
 - PV in q-major layout: out[q,dv] = P^T-slices @ [V|1] with F=65 (bf16),
   accumulated per 128-q-tile in PSUM [128,4,65]; col 64 = softmax denom.
   PSUM start marks the whole 2KB bank pending-zero, so only the first
   write into the bank carries start=True; each t-group's first write then
   self-zeroes its region
 - normalization per-partition (q on partitions): DVE reciprocal +
   tensor_scalar_mul; O[q,dv] -> O^T via DMA XBAR transpose (2 heads
   staged side by side); out-proj runs from O^T in bf16
 - PE-filler queue interleaves QKV/out-proj pieces between attention pairs
   so the PE never waits on the (Act-bound) exp stream
"""

import sys

if "/opt/trn_rl_repo" not in sys.path:
    sys.path.insert(0, "/opt/trn_rl_repo")

import ml_dtypes
import numpy as np

import concourse.bass as bass  # noqa: F401 (import registers engines)
import concourse.mybir as mybir
import concourse.tile as tile
from concourse import bacc
from concourse.bass_utils import run_bass_kernel_spmd

f32 = mybir.dt.float32
bf16 = mybir.dt.bfloat16
fp8 = mybir.dt.float8e4
DR = mybir.MatmulPerfMode.DoubleRow
AF = mybir.ActivationFunctionType
OP = mybir.AluOpType

D = 1024  # d_model
N = 2048  # n_ctx
HG = 256  # head-group width per core (4 heads x 64)
WS = 32.0  # fp8 weight pre-scale

# PE rows of filler emitted per attention pair (matches the ~2.1us the Act
# engine spends on the pair's two exps, minus the pair's own PE work)
FILL_ROWS = 2600


def make_tri() -> np.ndarray:
    """tri[kk, t, q'] = 1.0 if 128*t + kk <= q' else 0 — staircase masks."""
    kk = np.arange(128)[:, None, None]
    t = np.arange(4)[None, :, None]
    qp = np.arange(512)[None, None, :]
    return (128 * t + kk <= qp).astype(ml_dtypes.bfloat16)


def build_nc():
    nc = bacc.Bacc("TRN2", target_bir_lowering=False, debug=False)

    x8d = nc.declare_dram_parameter("x8", [D, N], fp8, isOutput=False)
    x8rd = nc.declare_dram_parameter("x8r", [D, N], fp8, isOutput=False)
    w8d = {}
    for w in ("q", "k", "v"):
        w8d[w, 0] = nc.declare_dram_parameter(f"w8{w}", [128, 8, HG], fp8, isOutput=False)
        w8d[w, 1] = nc.declare_dram_parameter(f"w8r{w}", [128, 8, HG], fp8, isOutput=False)
    bqd = nc.declare_dram_parameter("bq", [HG], f32, isOutput=False)
    bkd = nc.declare_dram_parameter("bk", [HG], f32, isOutput=False)
    bk8d = nc.declare_dram_parameter("bk8", [HG], fp8, isOutput=False)
    bvd = nc.declare_dram_parameter("bv", [HG], bf16, isOutput=False)  # pre-scaled x32
    wo = nc.declare_dram_parameter("wo", [HG, D], bf16, isOutput=False)
    trid = nc.declare_dram_parameter("tri", [128, 4, 512], bf16, isOutput=False)
    y = nc.declare_dram_parameter("y", [N, D], bf16, isOutput=True)

    x8_r = x8d[:].rearrange("(c p) n -> p c n", p=128)
    x8r_r = x8rd[:].rearrange("(c p) n -> p c n", p=128)

    with tile.TileContext(nc) as tc:
        with (
            tc.tile_pool(name="persist", bufs=1) as P1,
            tc.tile_pool(name="xtp", bufs=3) as XTP,
            tc.tile_pool(name="pp", bufs=6) as PP,
            tc.tile_pool(name="stg", bufs=4) as STG,
            tc.tile_pool(name="rp", bufs=8) as RP,
            tc.tile_pool(name="yp", bufs=6) as YP,
            tc.tile_pool(name="ps_s", bufs=2, space="PSUM") as PSA,
            tc.tile_pool(name="ps_pv", bufs=2, space="PSUM") as PVQ,
            tc.tile_pool(name="ps_b", bufs=2, space="PSUM") as PSB,
        ):
            # ---------- persistent tiles ----------
            w_sb = {}
            for w in ("q", "k", "v"):
                for r in (0, 1):
                    w_sb[w, r] = P1.tile([128, 8, HG], fp8, tag=f"w{w}{r}", name=f"w{w}{r}")
            wo_sb = P1.tile([128, 2, D], bf16, tag="wo")
            bq_sb = P1.tile([128, 2], f32, tag="bq")
            bk_sb = P1.tile([128, 2], f32, tag="bk")
            bv_sb = P1.tile([1, HG], bf16, tag="bv")
            ones_sb = P1.tile([1, 128], bf16, tag="ones")
            tri_sb = P1.tile([128, 4, 512], bf16, tag="tri")

            q8 = P1.tile([128, 2, N], fp8, tag="q8")
            k8 = P1.tile([128, 2, 2, N], fp8, tag="k8")
            bk8 = P1.tile([1, HG], fp8, tag="bk8")  # e4m3(16*bk): DR adds it twice
            ones8 = P1.tile([1, 512], fp8, tag="ones8")
            v1 = P1.tile([128, 16, 4, 65], bf16, tag="v1")
            att = P1.tile([128, 2, N], bf16, tag="att")

            dummy = P1.tile([1, 512], bf16, tag="dummy")
            nc.gpsimd.memset(dummy[:], 1.0)
            wps = PSB.tile([128, 512], f32, tag="b", name="warm")
            for i in range(10):
                nc.tensor.matmul(
                    wps[:], dummy[0:1, 0:128], dummy[0:1, :],
                    start=(i == 0), stop=(i == 9),
                )

            # ---------- initial loads ----------
            nc.sync.dma_start(w_sb["q", 0][:], w8d["q", 0][:])

            def emit_xt_load(n4, eng=None):
                eng = eng or nc.sync
                ns = slice(512 * n4, 512 * n4 + 512)
                tiles = []
                for i, (nm, src) in enumerate(
                    (("xa", x8_r), ("xb", x8_r), ("xar", x8r_r), ("xbr", x8r_r))
                ):
                    lo = 0 if nm in ("xa", "xar") else 4
                    t = XTP.tile([128, 4, 512], fp8, tag=nm, name=f"{nm}{n4}")
                    eng.dma_start(t[:], src[:, lo : lo + 4, ns])
                    tiles.append(t)
                return tiles  # [xa, xb, xar, xbr]

            xts = {0: emit_xt_load(0, eng=nc.sync)}
            nc.sync.dma_start(w_sb["q", 1][:], w8d["q", 1][:])
            nc.sync.dma_start(w_sb["k", 0][:], w8d["k", 0][:])
            nc.sync.dma_start(w_sb["k", 1][:], w8d["k", 1][:])
            nc.sync.dma_start(w_sb["v", 0][:], w8d["v", 0][:])
            nc.sync.dma_start(w_sb["v", 1][:], w8d["v", 1][:])
            nc.gpsimd.dma_start(bq_sb[:], bqd[:].rearrange("(m p) -> p m", p=128))
            nc.gpsimd.dma_start(bk_sb[:], bkd[:].rearrange("(m p) -> p m", p=128))
            nc.gpsimd.dma_start(bv_sb[:], bvd[None, :])
            nc.gpsimd.dma_start(bk8[:], bk8d[None, :])
            nc.gpsimd.memset(ones_sb[:], 1.0)
            nc.gpsimd.memset(ones8[:], 1.0)
            for h in range(4):
                nc.gpsimd.memset(v1[:, :, h, 64:65], 1.0)

            # ---------- QKV / out-proj piece emitters ----------
            def emit_qk_piece(which, m, n4):
                xa, xb, xar, xbr = xts[n4]
                ns = slice(512 * n4, 512 * n4 + 512)
                msl = slice(128 * m, 128 * m + 128)
                ps = PSB.tile([128, 512], f32, tag="b", name=f"{which}ps{m}{n4}")
                terms = ((0, (xa, xb)), (0, (xar, xbr)), (1, (xa, xb)))
                for ti, (wt, xp) in enumerate(terms):
                    wsb = w_sb[which, wt]
                    for hb in (0, 1):
                        for j in (0, 1):
                            c2 = 2 * (2 * hb + j)
                            nc.tensor.matmul(
                                ps[:],
                                wsb[:, c2 : c2 + 2, msl],
                                xp[hb][:, 2 * j : 2 * j + 2, :],
                                start=(ti == 0 and hb == 0 and j == 0),
                                stop=(which == "q" and ti == 2 and hb == 1 and j == 1),
                                perf_mode=DR,
                            )
                if which == "k":
                    # bias into psum (16*bk added via both DR subtiles), so the
                    # residual subtile compensates the k-side fp8 quantization
                    nc.tensor.matmul(
                        ps[:],
                        bk8[0:1, msl].unsqueeze(1).broadcast_to([1, 2, 128]),
                        ones8[0:1, :].unsqueeze(1).broadcast_to([1, 2, 512]),
                        start=False,
                        stop=True,
                        perf_mode=DR,
                    )
                    # these gate the next substream's scores: outrank y-copies
                    # and V-copies on the DVE heap
                    with tc.high_priority(offset=100):
                        nc.vector.tensor_scalar_mul(k8[:, m, 0, ns], ps[:], 1.0 / WS)
                        nc.vector.scalar_tensor_tensor(
                            k8[:, m, 1, ns], ps[:], 1.0 / WS, k8[:, m, 0, ns],
                            OP.mult, OP.subtract,
                        )
                else:
                    with tc.high_priority(offset=100):
                        nc.vector.tensor_scalar(
                            q8[:, m, ns], ps[:], 1.0 / WS, bq_sb[:, m : m + 1],
                            OP.mult, OP.add,
                        )

            def emit_v_piece(sch):
                xa, xb, xar, xbr = xts[sch // 4]
                so = 128 * (sch % 4)
                ps = PSB.tile([128, 256], f32, tag="b", name=f"vps{sch}")
                terms = ((0, (xa, xb)), (0, (xar, xbr)), (1, (xa, xb)))
                for ti, (wt, xp) in enumerate(terms):
                    wsb = w_sb["v", wt]
                    for hb in (0, 1):
                        for j in (0, 1):
                            c2 = 2 * (2 * hb + j)
                            nc.tensor.matmul(
                                ps[:],
                                xp[hb][:, 2 * j : 2 * j + 2, so : so + 128],
                                wsb[:, c2 : c2 + 2, :],
                                start=(ti == 0 and hb == 0 and j == 0),
                                stop=False,
                                perf_mode=DR,
                            )
                nc.tensor.matmul(ps[:], ones_sb[:], bv_sb[:], start=False, stop=True)
                nc.vector.tensor_scalar(
                    v1[:, sch, :, 0:64],
                    ps[:].rearrange("p (h d) -> p h d", h=4),
                    1.0 / WS,
                    None,
                    OP.mult,
                )

            yts = {}

            def emit_outproj_piece(T, dseg, tail=False):
                if dseg == 0:
                    yts[T] = YP.tile([128, D], bf16, tag="y", name=f"yt{T}")
                yt = yts[T]
                ps = PSB.tile([128, 512], f32, tag="b", name=f"yps{T}{dseg}")
                for hp in range(2):
                    nc.tensor.matmul(
                        ps[:],
                        att[:, hp, 128 * T : 128 * T + 128],
                        wo_sb[:, hp, 512 * dseg : 512 * dseg + 512],
                        start=(hp == 0),
                        stop=(hp == 1),
                    )
                dst = yt[:, 512 * dseg : 512 * dseg + 512]
                if tail and dseg == 0:
                    nc.scalar.copy(out=dst, in_=ps[:])
                else:
                    nc.vector.tensor_copy(out=dst, in_=ps[:])
                if tail:
                    nc.sync.dma_start(
                        y[128 * T : 128 * T + 128, 512 * dseg : 512 * dseg + 512], dst
                    )
                elif dseg == 1:
                    nc.sync.dma_start(y[128 * T : 128 * T + 128, :], yt[:])

            # ---------- PE filler queue ----------
            pending = []  # entries (key, rows, fn); key ('q'/'k', n4, m) / ('v', sch) / ('op', T)

            def push(key, rows, fn):
                pending.append((key, rows, fn))

            def require(pred):
                rest = []
                for e in pending:
                    if pred(e[0]):
                        e[2]()
                    else:
                        rest.append(e)
                pending[:] = rest

            def fill(budget):
                while pending and budget > 0:
                    key, rows, fn = pending.pop(0)
                    fn()
                    budget -= rows

            def push_qkv(n4, m_list=(0, 1)):
                for m in m_list:
                    push(("k", n4, m), 3072, lambda m=m: emit_qk_piece("k", m, n4))
                for sch in range(4 * n4, 4 * n4 + 4):
                    push(("v", sch, 0), 1792, lambda sch=sch: emit_v_piece(sch))
                for m in m_list:
                    push(("q", n4, m), 3072, lambda m=m: emit_qk_piece("q", m, n4))

            # ---------- attention substream ----------
            def emit_substream(half, s, hp, eager_tail=False):
                q0 = 1024 * half + 512 * s
                n4q = q0 // 512
                n_kc = 8 if half == 0 else 12 + 4 * s
                diag0 = 8 + 4 * s  # first diag kc (half 1 only)
                kc_max = [7 if half == 0 else 8 + 4 * s + t for t in range(4)]

                def off(kc):
                    return 128 * (kc - diag0) if (half == 1 and kc >= diag0) else 0

                require(lambda k: k[0] == "q" and k[1] == n4q and k[2] == hp)
                pvq = {
                    par: PVQ.tile(
                        [128, 4, 65], f32, tag="pvq", name=f"pvq{half}{s}{hp}{par}"
                    )
                    for par in (0, 1)
                }
                stage = STG.tile([128, 4, 128], bf16, tag="stg", name=f"stg{half}{s}{hp}")
                for p in range(n_kc // 2):
                    require(
                        lambda k, p=p: (
                            k[0] == "k" and k[1] <= (2 * p + 1) // 4 and k[2] == hp
                        )
                        or (k[0] == "v" and k[1] <= 2 * p + 1)
                    )
                    s_ps, pts = {}, {}
                    # score matmuls outrank recently-queued filler pieces on
                    # the PE heap: they are tiny and unblock the Act engine
                    with tc.high_priority(offset=100):
                        for par in (0, 1):
                            base = 64 * par
                            st = PSA.tile(
                                [128, 2, 512], f32, tag="s", name=f"s{half}{s}{hp}{par}{p}"
                            )
                            for j, kc in enumerate((2 * p, 2 * p + 1)):
                                o = off(kc)
                                nc.tensor.matmul(
                                    st[:, j, o:512],
                                    k8[base : base + 64, hp, :, 128 * kc : 128 * kc + 128],
                                    q8[base : base + 64, hp, q0 + o : q0 + 512]
                                    .unsqueeze(1)
                                    .broadcast_to([64, 2, 512 - o]),
                                    start=True,
                                    stop=True,
                                    perf_mode=DR,
                                )
                            s_ps[par] = st
                    for par in (0, 1):
                        pt = PP.tile(
                            [128, 2, 512], bf16, tag="p", name=f"p{half}{s}{hp}{par}{p}"
                        )
                        offs = [off(2 * p), off(2 * p + 1)]
                        if offs[1] == 0:
                            nc.scalar.activation(pt[:], s_ps[par][:], AF.Exp, scale=0.125)
                        else:
                            for j in (0, 1):
                                o = offs[j]
                                nc.scalar.activation(
                                    pt[:, j, o:512], s_ps[par][:, j, o:512], AF.Exp, scale=0.125
                                )
                        if half == 1:
                            for j, kc in enumerate((2 * p, 2 * p + 1)):
                                if kc >= diag0:
                                    tt = kc - diag0
                                    csl = slice(128 * tt, 128 * tt + 128)
                                    nc.vector.tensor_tensor(
                                        pt[:, j, csl], pt[:, j, csl], tri_sb[:, tt, csl], OP.mult
                                    )
                        pts[par] = pt
                    for par in (0, 1):
                        for t in range(4):
                            for j, kc in enumerate((2 * p, 2 * p + 1)):
                                if kc <= kc_max[t]:
                                    # start only on the bank's very first write:
                                    # it marks the whole 2KB bank pending-zero,
                                    # so each t-group's first write replaces
                                    # (self-zeroes) and later writes accumulate.
                                    nc.tensor.matmul(
                                        pvq[par][:, t, :],
                                        pts[par][:, j, 128 * t : 128 * t + 128],
                                        v1[:, kc, 2 * hp + par, :],
                                        start=(kc == 0 and t == 0),
                                        stop=(kc == kc_max[t]),
                                        skip_group_check=(not (kc == 0 and t == 0)),
                                    )
                    if eager_tail:
                        # finish q-tiles whose PV chains just stopped: norm,
                        # transpose, and out-proj inline so the kernel tail is
                        # a short per-tile pipeline instead of one serial drain
                        for t in range(4):
                            if kc_max[t] in (2 * p, 2 * p + 1):
                                for par in (0, 1):
                                    rc = RP.tile(
                                        [128, 1], f32, tag="rcp", name=f"rce{t}{par}"
                                    )
                                    nc.vector.reciprocal(rc[:], pvq[par][:, t, 64:65])
                                    nc.vector.tensor_scalar_mul(
                                        stage[:, t, 64 * par : 64 * par + 64],
                                        pvq[par][:, t, 0:64],
                                        rc[:, 0:1],
                                    )
                                nc.sync.dma_start(
                                    att[:, hp, q0 + 128 * t : q0 + 128 * t + 128],
                                    stage[:, t, :],
                                    transpose=True,
                                )
                                for dseg in range(2):
                                    emit_outproj_piece(8 * half + 4 * s + t, dseg)
                    fill(FILL_ROWS)
                if eager_tail:
                    return
                # normalize + transpose into att; these free the PVQ banks
                # the next substream's PV chains are waiting on
                with tc.high_priority(offset=150):
                    for par in (0, 1):
                        rcp = RP.tile([128, 4], f32, tag="rcp4", name=f"rcp{half}{s}{hp}{par}")
                        nc.vector.reciprocal(rcp[:], pvq[par][:, :, 64])
                        for t in range(4):
                            nc.vector.tensor_scalar_mul(
                                stage[:, t, 64 * par : 64 * par + 64],
                                pvq[par][:, t, 0:64],
                                rcp[:, t : t + 1],
                            )
                    for t in range(4):
                        nc.sync.dma_start(
                            att[:, hp, q0 + 128 * t : q0 + 128 * t + 128],
                            stage[:, t, :],
                            transpose=True,
                        )

            # ---------- main emission ----------
            emit_qk_piece("q", 0, 0)
            emit_qk_piece("k", 0, 0)
            for sch in range(4):
                emit_v_piece(sch)
            xts[1] = emit_xt_load(1)
            nc.sync.dma_start(tri_sb[:], trid[:])
            nc.sync.dma_start(wo_sb[:], wo[:].rearrange("(c p) n -> p c n", p=128))
            push(("k", 0, 1), 3072, lambda: emit_qk_piece("k", 1, 0))
            push(("q", 0, 1), 3072, lambda: emit_qk_piece("q", 1, 0))
            push_qkv(1)

            SUBS = [(h, s, hp) for h, s in ((0, 0), (0, 1), (1, 0), (1, 1)) for hp in (0, 1)]
            for i, (half, s, hp) in enumerate(SUBS):
                if (half, s, hp) == (0, 0, 0):
                    xts[2] = emit_xt_load(2)
                    push_qkv(2)
                if (half, s, hp) == (0, 1, 0):
                    xts[3] = emit_xt_load(3)
                    push_qkv(3)
                # pull the next substream's qT piece ahead of this substream's
                # DVE backlog so its first scores aren't gated on the bias-add
                if i + 1 < len(SUBS):
                    nh, ns_, nhp = SUBS[i + 1]
                    nn4q = (1024 * nh + 512 * ns_) // 512
                    require(lambda k, n=nn4q, m=nhp: k[0] == "q" and k[1] == n and k[2] == m)
                emit_substream(half, s, hp, eager_tail=(i == len(SUBS) - 1))
                if hp == 1 and (half, s) != (1, 1):
                    for t in range(4):
                        T = 8 * half + 4 * s + t
                        for dseg in range(2):
                            push(
                                ("op", T, dseg),
                                1024,
                                lambda T=T, d=dseg: emit_outproj_piece(T, d),
                            )

            require(lambda k: True)

    nc.compile()
    return nc


_NC = None
_TRI = None


def _get_nc():
    global _NC, _TRI
    if _NC is None:
        _NC = build_nc()
        _TRI = make_tri()
    return _NC


def make_in_maps(x, Wq, bq, Wk, bk, Wv, bv, Wo):
    _get_nc()
    bf = ml_dtypes.bfloat16
    e4 = mybir.dt.np(fp8)
    x = np.asarray(x, np.float32)
    in_maps = []
    for core in range(8):
        b, g = core // 4, core % 4
        sl = slice(HG * g, HG * (g + 1))
        xt = np.ascontiguousarray(x[b].T)
        x8 = xt.astype(e4)
        x8r = (xt - x8.astype(np.float32)).astype(e4)
        m = {
            "x8": x8,
            "x8r": x8r,
            "bq": np.ascontiguousarray(np.asarray(bq, np.float32)[sl]),
            "bk": np.ascontiguousarray(np.asarray(bk, np.float32)[sl]),
            "bk8": np.ascontiguousarray(16.0 * np.asarray(bk, np.float32)[sl]).astype(e4),
            "bv": np.ascontiguousarray(WS * np.asarray(bv, np.float32)[sl]).astype(bf),
            "wo": np.ascontiguousarray(np.asarray(Wo, np.float32)[sl, :].astype(bf)),
            "tri": _TRI,
        }
        for nm, W in (("q", Wq), ("k", Wk), ("v", Wv)):
            ws = WS * np.asarray(W, np.float32)[:, sl]
            w8 = ws.astype(e4)
            w8r = (ws - w8.astype(np.float32)).astype(e4)
            # pre-rearranged (c p) m -> p c m so the DMA moves 2KB descriptors
            m[f"w8{nm}"] = np.ascontiguousarray(
                w8.reshape(8, 128, HG).transpose(1, 0, 2)
            )
            m[f"w8r{nm}"] = np.ascontiguousarray(
                w8r.reshape(8, 128, HG).transpose(1, 0, 2)
            )
        in_maps.append(m)
    return in_maps


def kernel(x, Wq, bq, Wk, bk, Wv, bv, Wo, _trace=False, _trace_kwargs=None):
    nc = _get_nc()
    in_maps = make_in_maps(x, Wq, bq, Wk, bk, Wv, bv, Wo)
    res = run_bass_kernel_spmd(
        nc, in_maps, list(range(8)), trace=_trace, **(_trace_kwargs or {})
    )
    out = np.zeros((2, N, D), np.float64)
    for core in range(8):
        out[core // 4] += np.asarray(res.results[core]["y"]).astype(np.float64)
    y = out.astype(np.float32)
    if _trace:
        return y, res
    return y


# revision 44
# speedup vs baseline: 1.0296x; 1.0091x over previous
"""Sparse (half-causal) multi-head attention on 8 Trainium2 NeuronCores.

Problem: x[2,2048,1024] -> QKV proj (16 heads, dk=dv=64) -> scores with
half-causal mask (rows <1024 attend cols <1024 dense; rows >=1024 causal)
-> softmax -> out proj.

Sharding: 8 cores = 2 batches x 4 head-groups (4 heads each).  Each core
computes its batch's full QKV for its 4 heads (column-sharded W), attention
for those heads, and a partial output projection (row-sharded Wo).  Host
sums the 4 partials per batch.

v3 design:
 - QKV projections run as fp8e4 DoubleRow matmuls with residual
   compensation: W ~ (A + Ar)/32 and x ~ X + Xr, each term e4m3, giving
   bf16-level accuracy at 2x the f32r matmul rate (12 DR matmuls replace
   16 f32r-equivalent rows per piece); the 1/32 descale + bias fold into
   the psum->SBUF tensor_scalar
 - q-outer attention: per 512-wide q seg, scores S^T[k,q] (bf16 operands)
   land in PSUM [128,2,512] tiles, exp'd into bf16 p tiles; diag chunks
   compute/exp only the valid [128*t_tri, 512) column range
 - PV in q-major layout: out[q,dv] = P^T-slices @ [V|1] with F=65 (bf16),
   accumulated per 128-q-tile in PSUM [128,4,65]; col 64 = softmax denom.
   PSUM start marks the whole 2KB bank pending-zero, so only the first
   write into the bank carries start=True; each t-group's first write then
   self-zeroes its region
 - normalization per-partition (q on partitions): DVE reciprocal +
   tensor_scalar_mul; O[q,dv] -> O^T via DMA XBAR transpose (2 heads
   staged side by side); out-proj runs from O^T in bf16
 - PE-filler queue interleaves QKV/out-proj pieces between attention pairs
   so the PE never waits on the (Act-bound) exp stream
"""

import sys

if "/opt/trn_rl_repo" not in sys.path:
    sys.path.insert(0, "/opt/trn_rl_repo")

import ml_dtypes
import numpy as np

import concourse.bass as bass  # noqa: F401 (import registers engines)
import concourse.mybir as mybir
import concourse.tile as tile
from concourse import bacc
from concourse.bass_utils import run_bass_kernel_spmd

f32 = mybir.dt.float32
bf16 = mybir.dt.bfloat16
fp8 = mybir.dt.float8e4
DR = mybir.MatmulPerfMode.DoubleRow
AF = mybir.ActivationFunctionType
OP = mybir.AluOpType

D = 1024  # d_model
N = 2048  # n_ctx
HG = 256  # head-group width per core (4 heads x 64)
WS = 32.0  # fp8 weight pre-scale

# PE rows of filler emitted per attention pair (matches the ~2.1us the Act
# engine spends on the pair's two exps, minus the pair's own PE work)
FILL_ROWS = 2600


def make_tri() -> np.ndarray:
    """tri[kk, t, q'] = 1.0 if 128*t + kk <= q' else 0 — staircase masks."""
    kk = np.arange(128)[:, None, None]
    t = np.arange(4)[None, :, None]
    qp = np.arange(512)[None, None, :]
    return (128 * t + kk <= qp).astype(ml_dtypes.bfloat16)


def build_nc():
    nc = bacc.Bacc("TRN2", target_bir_lowering=False, debug=False)

    x8d = nc.declare_dram_parameter("x8", [D, N], fp8, isOutput=False)
    x8rd = nc.declare_dram_parameter("x8r", [D, N], fp8, isOutput=False)
    w8d = {}
    for w in ("q", "k", "v"):
        w8d[w, 0] = nc.declare_dram_parameter(f"w8{w}", [128, 8, HG], fp8, isOutput=False)
        w8d[w, 1] = nc.declare_dram_parameter(f"w8r{w}", [128, 8, HG], fp8, isOutput=False)
    bqd = nc.declare_dram_parameter("bq", [HG], f32, isOutput=False)
    bkd = nc.declare_dram_parameter("bk", [HG], f32, isOutput=False)
    bk8d = nc.declare_dram_parameter("bk8", [HG], fp8, isOutput=False)
    bvd = nc.declare_dram_parameter("bv", [HG], bf16, isOutput=False)  # pre-scaled x32
    wo = nc.declare_dram_parameter("wo", [HG, D], bf16, isOutput=False)
    trid = nc.declare_dram_parameter("tri", [128, 4, 512], bf16, isOutput=False)
    y = nc.declare_dram_parameter("y", [N, D], bf16, isOutput=True)

    x8_r = x8d[:].rearrange("(c p) n -> p c n", p=128)
    x8r_r = x8rd[:].rearrange("(c p) n -> p c n", p=128)

    with tile.TileContext(nc) as tc:
        with (
            tc.tile_pool(name="persist", bufs=1) as P1,
            tc.tile_pool(name="xtp", bufs=3) as XTP,
            tc.tile_pool(name="pp", bufs=6) as PP,
            tc.tile_pool(name="stg", bufs=4) as STG,
            tc.tile_pool(name="rp", bufs=8) as RP,
            tc.tile_pool(name="yp", bufs=6) as YP,
            tc.tile_pool(name="ps_s", bufs=2, space="PSUM") as PSA,
            tc.tile_pool(name="ps_pv", bufs=2, space="PSUM") as PVQ,
            tc.tile_pool(name="ps_b", bufs=2, space="PSUM") as PSB,
        ):
            # ---------- persistent tiles ----------
            w_sb = {}
            for w in ("q", "k", "v"):
                for r in (0, 1):
                    w_sb[w, r] = P1.tile([128, 8, HG], fp8, tag=f"w{w}{r}", name=f"w{w}{r}")
            wo_sb = P1.tile([128, 2, D], bf16, tag="wo")
            bq_sb = P1.tile([128, 2], f32, tag="bq")
            bk_sb = P1.tile([128, 2], f32, tag="bk")
            bv_sb = P1.tile([1, HG], bf16, tag="bv")
            ones_sb = P1.tile([1, 128], bf16, tag="ones")
            tri_sb = P1.tile([128, 4, 512], bf16, tag="tri")

            q8 = P1.tile([128, 2, N], fp8, tag="q8")
            k8 = P1.tile([128, 2, 2, N], fp8, tag="k8")
            bk8 = P1.tile([1, HG], fp8, tag="bk8")  # e4m3(16*bk): DR adds it twice
            ones8 = P1.tile([1, 512], fp8, tag="ones8")
            v1 = P1.tile([128, 16, 4, 65], bf16, tag="v1")
            att = P1.tile([128, 2, N], bf16, tag="att")

            dummy = P1.tile([1, 512], bf16, tag="dummy")
            nc.gpsimd.memset(dummy[:], 1.0)
            wps = PSB.tile([128, 512], f32, tag="b", name="warm")
            for i in range(10):
                nc.tensor.matmul(
                    wps[:], dummy[0:1, 0:128], dummy[0:1, :],
                    start=(i == 0), stop=(i == 9),
                )

            # ---------- initial loads ----------
            nc.sync.dma_start(w_sb["q", 0][:], w8d["q", 0][:])

            def emit_xt_load(n4, eng=None):
                eng = eng or nc.sync
                ns = slice(512 * n4, 512 * n4 + 512)
                tiles = []
                for i, (nm, src) in enumerate(
                    (("xa", x8_r), ("xb", x8_r), ("xar", x8r_r), ("xbr", x8r_r))
                ):
                    lo = 0 if nm in ("xa", "xar") else 4
                    t = XTP.tile([128, 4, 512], fp8, tag=nm, name=f"{nm}{n4}")
                    eng.dma_start(t[:], src[:, lo : lo + 4, ns])
                    tiles.append(t)
                return tiles  # [xa, xb, xar, xbr]

            xts = {0: emit_xt_load(0, eng=nc.sync)}
            nc.sync.dma_start(w_sb["q", 1][:], w8d["q", 1][:])
            nc.sync.dma_start(w_sb["k", 0][:], w8d["k", 0][:])
            nc.sync.dma_start(w_sb["k", 1][:], w8d["k", 1][:])
            nc.sync.dma_start(w_sb["v", 0][:], w8d["v", 0][:])
            nc.sync.dma_start(w_sb["v", 1][:], w8d["v", 1][:])
            nc.gpsimd.dma_start(bq_sb[:], bqd[:].rearrange("(m p) -> p m", p=128))
            nc.gpsimd.dma_start(bk_sb[:], bkd[:].rearrange("(m p) -> p m", p=128))
            nc.gpsimd.dma_start(bv_sb[:], bvd[None, :])
            nc.gpsimd.dma_start(bk8[:], bk8d[None, :])
            nc.gpsimd.memset(ones_sb[:], 1.0)
            nc.gpsimd.memset(ones8[:], 1.0)
            for h in range(4):
                nc.gpsimd.memset(v1[:, :, h, 64:65], 1.0)

            # ---------- QKV / out-proj piece emitters ----------
            def emit_qk_piece(which, m, n4):
                xa, xb, xar, xbr = xts[n4]
                ns = slice(512 * n4, 512 * n4 + 512)
                msl = slice(128 * m, 128 * m + 128)
                ps = PSB.tile([128, 512], f32, tag="b", name=f"{which}ps{m}{n4}")
                terms = ((0, (xa, xb)), (0, (xar, xbr)), (1, (xa, xb)))
                for ti, (wt, xp) in enumerate(terms):
                    wsb = w_sb[which, wt]
                    for hb in (0, 1):
                        for j in (0, 1):
                            c2 = 2 * (2 * hb + j)
                            nc.tensor.matmul(
                                ps[:],
                                wsb[:, c2 : c2 + 2, msl],
                                xp[hb][:, 2 * j : 2 * j + 2, :],
                                start=(ti == 0 and hb == 0 and j == 0),
                                stop=(which == "q" and ti == 2 and hb == 1 and j == 1),
                                perf_mode=DR,
                            )
                if which == "k":
                    # bias into psum (16*bk added via both DR subtiles), so the
                    # residual subtile compensates the k-side fp8 quantization
                    nc.tensor.matmul(
                        ps[:],
                        bk8[0:1, msl].unsqueeze(1).broadcast_to([1, 2, 128]),
                        ones8[0:1, :].unsqueeze(1).broadcast_to([1, 2, 512]),
                        start=False,
                        stop=True,
                        perf_mode=DR,
                    )
                    # these gate the next substream's scores: outrank y-copies
                    # and V-copies on the DVE heap
                    with tc.high_priority(offset=100):
                        nc.vector.tensor_scalar_mul(k8[:, m, 0, ns], ps[:], 1.0 / WS)
                        nc.vector.scalar_tensor_tensor(
                            k8[:, m, 1, ns], ps[:], 1.0 / WS, k8[:, m, 0, ns],
                            OP.mult, OP.subtract,
                        )
                else:
                    with tc.high_priority(offset=100):
                        nc.vector.tensor_scalar(
                            q8[:, m, ns], ps[:], 1.0 / WS, bq_sb[:, m : m + 1],
                            OP.mult, OP.add,
                        )

            def emit_v_piece(sch):
                xa, xb, xar, xbr = xts[sch // 4]
                so = 128 * (sch % 4)
                ps = PSB.tile([128, 256], f32, tag="b", name=f"vps{sch}")
                terms = ((0, (xa, xb)), (0, (xar, xbr)), (1, (xa, xb)))
                for ti, (wt, xp) in enumerate(terms):
                    wsb = w_sb["v", wt]
                    for hb in (0, 1):
                        for j in (0, 1):
                            c2 = 2 * (2 * hb + j)
                            nc.tensor.matmul(
                                ps[:],
                                xp[hb][:, 2 * j : 2 * j + 2, so : so + 128],
                                wsb[:, c2 : c2 + 2, :],
                                start=(ti == 0 and hb == 0 and j == 0),
                                stop=False,
                                perf_mode=DR,
                            )
                nc.tensor.matmul(ps[:], ones_sb[:], bv_sb[:], start=False, stop=True)
                nc.vector.tensor_scalar(
                    v1[:, sch, :, 0:64],
                    ps[:].rearrange("p (h d) -> p h d", h=4),
                    1.0 / WS,
                    None,
                    OP.mult,
                )

            yts = {}

            def emit_outproj_piece(T, dseg, tail=False):
                if dseg == 0:
                    yts[T] = YP.tile([128, D], bf16, tag="y", name=f"yt{T}")
                yt = yts[T]
                ps = PSB.tile([128, 512], f32, tag="b", name=f"yps{T}{dseg}")
                for hp in range(2):
                    nc.tensor.matmul(
                        ps[:],
                        att[:, hp, 128 * T : 128 * T + 128],
                        wo_sb[:, hp, 512 * dseg : 512 * dseg + 512],
                        start=(hp == 0),
                        stop=(hp == 1),
                    )
                dst = yt[:, 512 * dseg : 512 * dseg + 512]
                if tail and dseg == 0:
                    nc.scalar.copy(out=dst, in_=ps[:])
                else:
                    nc.vector.tensor_copy(out=dst, in_=ps[:])
                if tail:
                    nc.sync.dma_start(
                        y[128 * T : 128 * T + 128, 512 * dseg : 512 * dseg + 512], dst
                    )
                elif dseg == 1:
                    nc.sync.dma_start(y[128 * T : 128 * T + 128, :], yt[:])

            # ---------- PE filler queue ----------
            pending = []  # entries (key, rows, fn); key ('q'/'k', n4, m) / ('v', sch) / ('op', T)

            def push(key, rows, fn):
                pending.append((key, rows, fn))

            def require(pred):
                rest = []
                for e in pending:
                    if pred(e[0]):
                        e[2]()
                    else:
                        rest.append(e)
                pending[:] = rest

            def fill(budget):
                while pending and budget > 0:
                    key, rows, fn = pending.pop(0)
                    fn()
                    budget -= rows

            def push_qkv(n4, m_list=(0, 1)):
                for m in m_list:
                    push(("k", n4, m), 3072, lambda m=m: emit_qk_piece("k", m, n4))
                for sch in range(4 * n4, 4 * n4 + 4):
                    push(("v", sch, 0), 1792, lambda sch=sch: emit_v_piece(sch))
                for m in m_list:
                    push(("q", n4, m), 3072, lambda m=m: emit_qk_piece("q", m, n4))

            # ---------- attention substream ----------
            def emit_substream(half, s, hp, eager_tail=False):
                q0 = 1024 * half + 512 * s
                n4q = q0 // 512
                n_kc = 8 if half == 0 else 12 + 4 * s
                diag0 = 8 + 4 * s  # first diag kc (half 1 only)
                kc_max = [7 if half == 0 else 8 + 4 * s + t for t in range(4)]

                def off(kc):
                    return 128 * (kc - diag0) if (half == 1 and kc >= diag0) else 0

                require(lambda k: k[0] == "q" and k[1] == n4q and k[2] == hp)
                pvq = {
                    par: PVQ.tile(
                        [128, 4, 65], f32, tag="pvq", name=f"pvq{half}{s}{hp}{par}"
                    )
                    for par in (0, 1)
                }
                stage = STG.tile([128, 4, 128], bf16, tag="stg", name=f"stg{half}{s}{hp}")
                for p in range(n_kc // 2):
                    require(
                        lambda k, p=p: (
                            k[0] == "k" and k[1] <= (2 * p + 1) // 4 and k[2] == hp
                        )
                        or (k[0] == "v" and k[1] <= 2 * p + 1)
                    )
                    s_ps, pts = {}, {}
                    # score matmuls outrank recently-queued filler pieces on
                    # the PE heap: they are tiny and unblock the Act engine
                    with tc.high_priority(offset=100):
                        for par in (0, 1):
                            base = 64 * par
                            st = PSA.tile(
                                [128, 2, 512], f32, tag="s", name=f"s{half}{s}{hp}{par}{p}"
                            )
                            for j, kc in enumerate((2 * p, 2 * p + 1)):
                                o = off(kc)
                                nc.tensor.matmul(
                                    st[:, j, o:512],
                                    k8[base : base + 64, hp, :, 128 * kc : 128 * kc + 128],
                                    q8[base : base + 64, hp, q0 + o : q0 + 512]
                                    .unsqueeze(1)
                                    .broadcast_to([64, 2, 512 - o]),
                                    start=True,
                                    stop=True,
                                    perf_mode=DR,
                                )
                            s_ps[par] = st
                    for par in (0, 1):
                        pt = PP.tile(
                            [128, 2, 512], bf16, tag="p", name=f"p{half}{s}{hp}{par}{p}"
                        )
                        offs = [off(2 * p), off(2 * p + 1)]
                        if offs[1] == 0:
                            nc.scalar.activation(pt[:], s_ps[par][:], AF.Exp, scale=0.125)
                        else:
                            for j in (0, 1):
                                o = offs[j]
                                nc.scalar.activation(
                                    pt[:, j, o:512], s_ps[par][:, j, o:512], AF.Exp, scale=0.125
                                )
                        if half == 1:
                            for j, kc in enumerate((2 * p, 2 * p + 1)):
                                if kc >= diag0:
                                    tt = kc - diag0
                                    csl = slice(128 * tt, 128 * tt + 128)
                                    nc.vector.tensor_tensor(
                                        pt[:, j, csl], pt[:, j, csl], tri_sb[:, tt, csl], OP.mult
                                    )
                        pts[par] = pt
                    for par in (0, 1):
                        with tc.high_priority(offset=100):
                         for t in range(4):
                            for j, kc in enumerate((2 * p, 2 * p + 1)):
                                if kc <= kc_max[t]:
                                    # start only on the bank's very first write:
                                    # it marks the whole 2KB bank pending-zero,
                                    # so each t-group's first write replaces
                                    # (self-zeroes) and later writes accumulate.
                                    nc.tensor.matmul(
                                        pvq[par][:, t, :],
                                        pts[par][:, j, 128 * t : 128 * t + 128],
                                        v1[:, kc, 2 * hp + par, :],
                                        start=(kc == 0 and t == 0),
                                        stop=(kc == kc_max[t]),
                                        skip_group_check=(not (kc == 0 and t == 0)),
                                    )
                    if eager_tail:
                        # finish q-tiles whose PV chains just stopped: norm,
                        # transpose, and out-proj inline so the kernel tail is
                        # a short per-tile pipeline instead of one serial drain
                        for t in range(4):
                            if kc_max[t] in (2 * p, 2 * p + 1):
                                for par in (0, 1):
                                    rc = RP.tile(
                                        [128, 1], f32, tag="rcp", name=f"rce{t}{par}"
                                    )
                                    nc.vector.reciprocal(rc[:], pvq[par][:, t, 64:65])
                                    nc.vector.tensor_scalar_mul(
                                        stage[:, t, 64 * par : 64 * par + 64],
                                        pvq[par][:, t, 0:64],
                                        rc[:, 0:1],
                                    )
                                nc.sync.dma_start(
                                    att[:, hp, q0 + 128 * t : q0 + 128 * t + 128],
                                    stage[:, t, :],
                                    transpose=True,
                                )
                                for dseg in range(2):
                                    emit_outproj_piece(8 * half + 4 * s + t, dseg)
                    fill(FILL_ROWS)
                if eager_tail:
                    return
                # normalize + transpose into att; these free the PVQ banks
                # the next substream's PV chains are waiting on
                with tc.high_priority(offset=150):
                    for par in (0, 1):
                        rcp = RP.tile([128, 4], f32, tag="rcp4", name=f"rcp{half}{s}{hp}{par}")
                        nc.vector.reciprocal(rcp[:], pvq[par][:, :, 64])
                        for t in range(4):
                            nc.vector.tensor_scalar_mul(
                                stage[:, t, 64 * par : 64 * par + 64],
                                pvq[par][:, t, 0:64],
                                rcp[:, t : t + 1],
                            )
                    for t in range(4):
                        nc.sync.dma_start(
                            att[:, hp, q0 + 128 * t : q0 + 128 * t + 128],
                            stage[:, t, :],
                            transpose=True,
                        )

            # ---------- main emission ----------
            emit_qk_piece("q", 0, 0)
            emit_qk_piece("k", 0, 0)
            for sch in range(4):
                emit_v_piece(sch)
            xts[1] = emit_xt_load(1)
            nc.sync.dma_start(tri_sb[:], trid[:])
            nc.sync.dma_start(wo_sb[:], wo[:].rearrange("(c p) n -> p c n", p=128))
            push(("k", 0, 1), 3072, lambda: emit_qk_piece("k", 1, 0))
            push(("q", 0, 1), 3072, lambda: emit_qk_piece("q", 1, 0))
            push_qkv(1)

            SUBS = [(h, s, hp) for h, s in ((0, 0), (0, 1), (1, 0), (1, 1)) for hp in (0, 1)]
            for i, (half, s, hp) in enumerate(SUBS):
                if (half, s, hp) == (0, 0, 0):
                    xts[2] = emit_xt_load(2)
                    push_qkv(2)
                if (half, s, hp) == (0, 1, 0):
                    xts[3] = emit_xt_load(3)
                    push_qkv(3)
                # pull the next substream's qT piece ahead of this substream's
                # DVE backlog so its first scores aren't gated on the bias-add
                if i + 1 < len(SUBS):
                    nh, ns_, nhp = SUBS[i + 1]
                    nn4q = (1024 * nh + 512 * ns_) // 512
                    require(lambda k, n=nn4q, m=nhp: k[0] == "q" and k[1] == n and k[2] == m)
                emit_substream(half, s, hp, eager_tail=(i == len(SUBS) - 1))
                if hp == 1 and (half, s) != (1, 1):
                    for t in range(4):
                        T = 8 * half + 4 * s + t
                        for dseg in range(2):
                            push(
                                ("op", T, dseg),
                                1024,
                                lambda T=T, d=dseg: emit_outproj_piece(T, d),
                            )

            require(lambda k: True)

    nc.compile()
    return nc


_NC = None
_TRI = None


def _get_nc():
    global _NC, _TRI
    if _NC is None:
        _NC = build_nc()
        _TRI = make_tri()
    return _NC


def make_in_maps(x, Wq, bq, Wk, bk, Wv, bv, Wo):
    _get_nc()
    bf = ml_dtypes.bfloat16
    e4 = mybir.dt.np(fp8)
    x = np.asarray(x, np.float32)
    in_maps = []
    for core in range(8):
        b, g = core // 4, core % 4
        sl = slice(HG * g, HG * (g + 1))
        xt = np.ascontiguousarray(x[b].T)
        x8 = xt.astype(e4)
        x8r = (xt - x8.astype(np.float32)).astype(e4)
        m = {
            "x8": x8,
            "x8r": x8r,
            "bq": np.ascontiguousarray(np.asarray(bq, np.float32)[sl]),
            "bk": np.ascontiguousarray(np.asarray(bk, np.float32)[sl]),
            "bk8": np.ascontiguousarray(16.0 * np.asarray(bk, np.float32)[sl]).astype(e4),
            "bv": np.ascontiguousarray(WS * np.asarray(bv, np.float32)[sl]).astype(bf),
            "wo": np.ascontiguousarray(np.asarray(Wo, np.float32)[sl, :].astype(bf)),
            "tri": _TRI,
        }
        for nm, W in (("q", Wq), ("k", Wk), ("v", Wv)):
            ws = WS * np.asarray(W, np.float32)[:, sl]
            w8 = ws.astype(e4)
            w8r = (ws - w8.astype(np.float32)).astype(e4)
            # pre-rearranged (c p) m -> p c m so the DMA moves 2KB descriptors
            m[f"w8{nm}"] = np.ascontiguousarray(
                w8.reshape(8, 128, HG).transpose(1, 0, 2)
            )
            m[f"w8r{nm}"] = np.ascontiguousarray(
                w8r.reshape(8, 128, HG).transpose(1, 0, 2)
            )
        in_maps.append(m)
    return in_maps


def kernel(x, Wq, bq, Wk, bk, Wv, bv, Wo, _trace=False, _trace_kwargs=None):
    nc = _get_nc()
    in_maps = make_in_maps(x, Wq, bq, Wk, bk, Wv, bv, Wo)
    res = run_bass_kernel_spmd(
        nc, in_maps, list(range(8)), trace=_trace, **(_trace_kwargs or {})
    )
    out = np.zeros((2, N, D), np.float64)
    for core in range(8):
        out[core // 4] += np.asarray(res.results[core]["y"]).astype(np.float64)
    y = out.astype(np.float32)
    if _trace:
        return y, res
    return y


# revision 45
# speedup vs baseline: 1.0347x; 1.0050x over previous
"""Sparse (half-causal) multi-head attention on 8 Trainium2 NeuronCores.

Problem: x[2,2048,1024] -> QKV proj (16 heads, dk=dv=64) -> scores with
half-causal mask (rows <1024 attend cols <1024 dense; rows >=1024 causal)
-> softmax -> out proj.

Sharding: 8 cores = 2 batches x 4 head-groups (4 heads each).  Each core
computes its batch's full QKV for its 4 heads (column-sharded W), attention
for those heads, and a partial output projection (row-sharded Wo).  Host
sums the 4 partials per batch.

v3 design:
 - QKV projections run as fp8e4 DoubleRow matmuls with residual
   compensation: W ~ (A + Ar)/32 and x ~ X + Xr, each term e4m3, giving
   bf16-level accuracy at 2x the f32r matmul rate (12 DR matmuls replace
   16 f32r-equivalent rows per piece); the 1/32 descale + bias fold into
   the psum->SBUF tensor_scalar
 - q-outer attention: per 512-wide q seg, scores S^T[k,q] (bf16 operands)
   land in PSUM [128,2,512] tiles, exp'd into bf16 p tiles; diag chunks
   compute/exp only the valid [128*t_tri, 512) column range
 - PV in q-major layout: out[q,dv] = P^T-slices @ [V|1] with F=65 (bf16),
   accumulated per 128-q-tile in PSUM [128,4,65]; col 64 = softmax denom.
   PSUM start marks the whole 2KB bank pending-zero, so only the first
   write into the bank carries start=True; each t-group's first write then
   self-zeroes its region
 - normalization per-partition (q on partitions): DVE reciprocal +
   tensor_scalar_mul; O[q,dv] -> O^T via DMA XBAR transpose (2 heads
   staged side by side); out-proj runs from O^T in bf16
 - PE-filler queue interleaves QKV/out-proj pieces between attention pairs
   so the PE never waits on the (Act-bound) exp stream
"""

import sys

if "/opt/trn_rl_repo" not in sys.path:
    sys.path.insert(0, "/opt/trn_rl_repo")

import ml_dtypes
import numpy as np

import concourse.bass as bass  # noqa: F401 (import registers engines)
import concourse.mybir as mybir
import concourse.tile as tile
from concourse import bacc
from concourse.bass_utils import run_bass_kernel_spmd

f32 = mybir.dt.float32
bf16 = mybir.dt.bfloat16
fp8 = mybir.dt.float8e4
DR = mybir.MatmulPerfMode.DoubleRow
AF = mybir.ActivationFunctionType
OP = mybir.AluOpType

D = 1024  # d_model
N = 2048  # n_ctx
HG = 256  # head-group width per core (4 heads x 64)
WS = 32.0  # fp8 weight pre-scale

# PE rows of filler emitted per attention pair (matches the ~2.1us the Act
# engine spends on the pair's two exps, minus the pair's own PE work)
FILL_ROWS = 2600


def make_tri() -> np.ndarray:
    """tri[kk, t, q'] = 1.0 if 128*t + kk <= q' else 0 — staircase masks."""
    kk = np.arange(128)[:, None, None]
    t = np.arange(4)[None, :, None]
    qp = np.arange(512)[None, None, :]
    return (128 * t + kk <= qp).astype(ml_dtypes.bfloat16)


def build_nc():
    nc = bacc.Bacc("TRN2", target_bir_lowering=False, debug=False)

    x8d = nc.declare_dram_parameter("x8", [D, N], fp8, isOutput=False)
    x8rd = nc.declare_dram_parameter("x8r", [D, N], fp8, isOutput=False)
    w8d = {}
    for w in ("q", "k", "v"):
        w8d[w, 0] = nc.declare_dram_parameter(f"w8{w}", [128, 8, HG], fp8, isOutput=False)
        w8d[w, 1] = nc.declare_dram_parameter(f"w8r{w}", [128, 8, HG], fp8, isOutput=False)
    bqd = nc.declare_dram_parameter("bq", [HG], f32, isOutput=False)
    bkd = nc.declare_dram_parameter("bk", [HG], f32, isOutput=False)
    bk8d = nc.declare_dram_parameter("bk8", [HG], fp8, isOutput=False)
    bvd = nc.declare_dram_parameter("bv", [HG], bf16, isOutput=False)  # pre-scaled x32
    wo = nc.declare_dram_parameter("wo", [HG, D], bf16, isOutput=False)
    trid = nc.declare_dram_parameter("tri", [128, 4, 512], bf16, isOutput=False)
    y = nc.declare_dram_parameter("y", [N, D], bf16, isOutput=True)

    x8_r = x8d[:].rearrange("(c p) n -> p c n", p=128)
    x8r_r = x8rd[:].rearrange("(c p) n -> p c n", p=128)

    with tile.TileContext(nc) as tc:
        with (
            tc.tile_pool(name="persist", bufs=1) as P1,
            tc.tile_pool(name="xtp", bufs=3) as XTP,
            tc.tile_pool(name="pp", bufs=6) as PP,
            tc.tile_pool(name="stg", bufs=4) as STG,
            tc.tile_pool(name="rp", bufs=8) as RP,
            tc.tile_pool(name="yp", bufs=6) as YP,
            tc.tile_pool(name="ps_s", bufs=2, space="PSUM") as PSA,
            tc.tile_pool(name="ps_pv", bufs=2, space="PSUM") as PVQ,
            tc.tile_pool(name="ps_b", bufs=2, space="PSUM") as PSB,
        ):
            # ---------- persistent tiles ----------
            w_sb = {}
            for w in ("q", "k", "v"):
                for r in (0, 1):
                    w_sb[w, r] = P1.tile([128, 8, HG], fp8, tag=f"w{w}{r}", name=f"w{w}{r}")
            wo_sb = P1.tile([128, 2, D], bf16, tag="wo")
            bq_sb = P1.tile([128, 2], f32, tag="bq")
            bk_sb = P1.tile([128, 2], f32, tag="bk")
            bv_sb = P1.tile([1, HG], bf16, tag="bv")
            ones_sb = P1.tile([1, 128], bf16, tag="ones")
            tri_sb = P1.tile([128, 4, 512], bf16, tag="tri")

            q8 = P1.tile([128, 2, N], fp8, tag="q8")
            k8 = P1.tile([128, 2, 2, N], fp8, tag="k8")
            bk8 = P1.tile([1, HG], fp8, tag="bk8")  # e4m3(16*bk): DR adds it twice
            ones8 = P1.tile([1, 512], fp8, tag="ones8")
            v1 = P1.tile([128, 16, 4, 65], bf16, tag="v1")
            att = P1.tile([128, 2, N], bf16, tag="att")

            dummy = P1.tile([1, 512], bf16, tag="dummy")
            nc.gpsimd.memset(dummy[:], 1.0)
            wps = PSB.tile([128, 512], f32, tag="b", name="warm")
            for i in range(10):
                nc.tensor.matmul(
                    wps[:], dummy[0:1, 0:128], dummy[0:1, :],
                    start=(i == 0), stop=(i == 9),
                )

            # ---------- initial loads ----------
            nc.sync.dma_start(w_sb["q", 0][:], w8d["q", 0][:])

            def emit_xt_load(n4, eng=None):
                eng = eng or nc.sync
                ns = slice(512 * n4, 512 * n4 + 512)
                tiles = []
                for i, (nm, src) in enumerate(
                    (("xa", x8_r), ("xb", x8_r), ("xar", x8r_r), ("xbr", x8r_r))
                ):
                    lo = 0 if nm in ("xa", "xar") else 4
                    t = XTP.tile([128, 4, 512], fp8, tag=nm, name=f"{nm}{n4}")
                    eng.dma_start(t[:], src[:, lo : lo + 4, ns])
                    tiles.append(t)
                return tiles  # [xa, xb, xar, xbr]

            xts = {0: emit_xt_load(0, eng=nc.sync)}
            nc.sync.dma_start(w_sb["q", 1][:], w8d["q", 1][:])
            nc.sync.dma_start(w_sb["k", 0][:], w8d["k", 0][:])
            nc.sync.dma_start(w_sb["k", 1][:], w8d["k", 1][:])
            nc.sync.dma_start(w_sb["v", 0][:], w8d["v", 0][:])
            nc.sync.dma_start(w_sb["v", 1][:], w8d["v", 1][:])
            nc.gpsimd.dma_start(bq_sb[:], bqd[:].rearrange("(m p) -> p m", p=128))
            nc.gpsimd.dma_start(bk_sb[:], bkd[:].rearrange("(m p) -> p m", p=128))
            nc.gpsimd.dma_start(bv_sb[:], bvd[None, :])
            nc.gpsimd.dma_start(bk8[:], bk8d[None, :])
            nc.gpsimd.memset(ones_sb[:], 1.0)
            nc.gpsimd.memset(ones8[:], 1.0)
            for h in range(4):
                nc.gpsimd.memset(v1[:, :, h, 64:65], 1.0)

            # ---------- QKV / out-proj piece emitters ----------
            def emit_qk_piece(which, m, n4):
                xa, xb, xar, xbr = xts[n4]
                ns = slice(512 * n4, 512 * n4 + 512)
                msl = slice(128 * m, 128 * m + 128)
                ps = PSB.tile([128, 512], f32, tag="b", name=f"{which}ps{m}{n4}")
                terms = ((0, (xa, xb)), (0, (xar, xbr)), (1, (xa, xb)))
                for ti, (wt, xp) in enumerate(terms):
                    wsb = w_sb[which, wt]
                    for hb in (0, 1):
                        for j in (0, 1):
                            c2 = 2 * (2 * hb + j)
                            nc.tensor.matmul(
                                ps[:],
                                wsb[:, c2 : c2 + 2, msl],
                                xp[hb][:, 2 * j : 2 * j + 2, :],
                                start=(ti == 0 and hb == 0 and j == 0),
                                stop=(which == "q" and ti == 2 and hb == 1 and j == 1),
                                perf_mode=DR,
                            )
                if which == "k":
                    # bias into psum (16*bk added via both DR subtiles), so the
                    # residual subtile compensates the k-side fp8 quantization
                    nc.tensor.matmul(
                        ps[:],
                        bk8[0:1, msl].unsqueeze(1).broadcast_to([1, 2, 128]),
                        ones8[0:1, :].unsqueeze(1).broadcast_to([1, 2, 512]),
                        start=False,
                        stop=True,
                        perf_mode=DR,
                    )
                    # these gate the next substream's scores: outrank y-copies
                    # and V-copies on the DVE heap
                    with tc.high_priority(offset=100):
                        nc.vector.tensor_scalar_mul(k8[:, m, 0, ns], ps[:], 1.0 / WS)
                        nc.vector.scalar_tensor_tensor(
                            k8[:, m, 1, ns], ps[:], 1.0 / WS, k8[:, m, 0, ns],
                            OP.mult, OP.subtract,
                        )
                else:
                    with tc.high_priority(offset=100):
                        nc.vector.tensor_scalar(
                            q8[:, m, ns], ps[:], 1.0 / WS, bq_sb[:, m : m + 1],
                            OP.mult, OP.add,
                        )

            def emit_v_piece(sch):
                xa, xb, xar, xbr = xts[sch // 4]
                so = 128 * (sch % 4)
                ps = PSB.tile([128, 256], f32, tag="b", name=f"vps{sch}")
                terms = ((0, (xa, xb)), (0, (xar, xbr)), (1, (xa, xb)))
                for ti, (wt, xp) in enumerate(terms):
                    wsb = w_sb["v", wt]
                    for hb in (0, 1):
                        for j in (0, 1):
                            c2 = 2 * (2 * hb + j)
                            nc.tensor.matmul(
                                ps[:],
                                xp[hb][:, 2 * j : 2 * j + 2, so : so + 128],
                                wsb[:, c2 : c2 + 2, :],
                                start=(ti == 0 and hb == 0 and j == 0),
                                stop=False,
                                perf_mode=DR,
                            )
                nc.tensor.matmul(ps[:], ones_sb[:], bv_sb[:], start=False, stop=True)
                nc.vector.tensor_scalar(
                    v1[:, sch, :, 0:64],
                    ps[:].rearrange("p (h d) -> p h d", h=4),
                    1.0 / WS,
                    None,
                    OP.mult,
                )

            yts = {}

            def emit_outproj_piece(T, dseg, tail=False):
                if dseg == 0:
                    yts[T] = YP.tile([128, D], bf16, tag="y", name=f"yt{T}")
                yt = yts[T]
                ps = PSB.tile([128, 512], f32, tag="b", name=f"yps{T}{dseg}")
                for hp in range(2):
                    nc.tensor.matmul(
                        ps[:],
                        att[:, hp, 128 * T : 128 * T + 128],
                        wo_sb[:, hp, 512 * dseg : 512 * dseg + 512],
                        start=(hp == 0),
                        stop=(hp == 1),
                    )
                dst = yt[:, 512 * dseg : 512 * dseg + 512]
                if tail and dseg == 0:
                    nc.scalar.copy(out=dst, in_=ps[:])
                else:
                    nc.vector.tensor_copy(out=dst, in_=ps[:])
                if tail:
                    nc.sync.dma_start(
                        y[128 * T : 128 * T + 128, 512 * dseg : 512 * dseg + 512], dst
                    )
                elif dseg == 1:
                    nc.sync.dma_start(y[128 * T : 128 * T + 128, :], yt[:])

            # ---------- PE filler queue ----------
            pending = []  # entries (key, rows, fn); key ('q'/'k', n4, m) / ('v', sch) / ('op', T)

            def push(key, rows, fn):
                pending.append((key, rows, fn))

            def require(pred):
                # required pieces gate upcoming scores: rank them above
                # previously-queued background filler
                rest = []
                for e in pending:
                    if pred(e[0]):
                        with tc.high_priority(offset=100):
                            e[2]()
                    else:
                        rest.append(e)
                pending[:] = rest

            def fill(budget):
                while pending and budget > 0:
                    key, rows, fn = pending.pop(0)
                    fn()
                    budget -= rows

            def push_qkv(n4, m_list=(0, 1)):
                for m in m_list:
                    push(("k", n4, m), 3072, lambda m=m: emit_qk_piece("k", m, n4))
                for sch in range(4 * n4, 4 * n4 + 4):
                    push(("v", sch, 0), 1792, lambda sch=sch: emit_v_piece(sch))
                for m in m_list:
                    push(("q", n4, m), 3072, lambda m=m: emit_qk_piece("q", m, n4))

            # ---------- attention substream ----------
            def emit_substream(half, s, hp, eager_tail=False):
                q0 = 1024 * half + 512 * s
                n4q = q0 // 512
                n_kc = 8 if half == 0 else 12 + 4 * s
                diag0 = 8 + 4 * s  # first diag kc (half 1 only)
                kc_max = [7 if half == 0 else 8 + 4 * s + t for t in range(4)]

                def off(kc):
                    return 128 * (kc - diag0) if (half == 1 and kc >= diag0) else 0

                require(lambda k: k[0] == "q" and k[1] == n4q and k[2] == hp)
                pvq = {
                    par: PVQ.tile(
                        [128, 4, 65], f32, tag="pvq", name=f"pvq{half}{s}{hp}{par}"
                    )
                    for par in (0, 1)
                }
                stage = STG.tile([128, 4, 128], bf16, tag="stg", name=f"stg{half}{s}{hp}")
                for p in range(n_kc // 2):
                    require(
                        lambda k, p=p: (
                            k[0] == "k" and k[1] <= (2 * p + 1) // 4 and k[2] == hp
                        )
                        or (k[0] == "v" and k[1] <= 2 * p + 1)
                    )
                    s_ps, pts = {}, {}
                    # score matmuls outrank recently-queued filler pieces on
                    # the PE heap: they are tiny and unblock the Act engine
                    with tc.high_priority(offset=100):
                        for par in (0, 1):
                            base = 64 * par
                            st = PSA.tile(
                                [128, 2, 512], f32, tag="s", name=f"s{half}{s}{hp}{par}{p}"
                            )
                            for j, kc in enumerate((2 * p, 2 * p + 1)):
                                o = off(kc)
                                nc.tensor.matmul(
                                    st[:, j, o:512],
                                    k8[base : base + 64, hp, :, 128 * kc : 128 * kc + 128],
                                    q8[base : base + 64, hp, q0 + o : q0 + 512]
                                    .unsqueeze(1)
                                    .broadcast_to([64, 2, 512 - o]),
                                    start=True,
                                    stop=True,
                                    perf_mode=DR,
                                )
                            s_ps[par] = st
                    for par in (0, 1):
                        pt = PP.tile(
                            [128, 2, 512], bf16, tag="p", name=f"p{half}{s}{hp}{par}{p}"
                        )
                        offs = [off(2 * p), off(2 * p + 1)]
                        if offs[1] == 0:
                            nc.scalar.activation(pt[:], s_ps[par][:], AF.Exp, scale=0.125)
                        else:
                            for j in (0, 1):
                                o = offs[j]
                                nc.scalar.activation(
                                    pt[:, j, o:512], s_ps[par][:, j, o:512], AF.Exp, scale=0.125
                                )
                        if half == 1:
                            for j, kc in enumerate((2 * p, 2 * p + 1)):
                                if kc >= diag0:
                                    tt = kc - diag0
                                    csl = slice(128 * tt, 128 * tt + 128)
                                    nc.vector.tensor_tensor(
                                        pt[:, j, csl], pt[:, j, csl], tri_sb[:, tt, csl], OP.mult
                                    )
                        pts[par] = pt
                    for par in (0, 1):
                        with tc.high_priority(offset=100):
                         for t in range(4):
                            for j, kc in enumerate((2 * p, 2 * p + 1)):
                                if kc <= kc_max[t]:
                                    # start only on the bank's very first write:
                                    # it marks the whole 2KB bank pending-zero,
                                    # so each t-group's first write replaces
                                    # (self-zeroes) and later writes accumulate.
                                    nc.tensor.matmul(
                                        pvq[par][:, t, :],
                                        pts[par][:, j, 128 * t : 128 * t + 128],
                                        v1[:, kc, 2 * hp + par, :],
                                        start=(kc == 0 and t == 0),
                                        stop=(kc == kc_max[t]),
                                        skip_group_check=(not (kc == 0 and t == 0)),
                                    )
                    if eager_tail:
                        # finish q-tiles whose PV chains just stopped: norm,
                        # transpose, and out-proj inline so the kernel tail is
                        # a short per-tile pipeline instead of one serial drain
                        for t in range(4):
                            if kc_max[t] in (2 * p, 2 * p + 1):
                                for par in (0, 1):
                                    rc = RP.tile(
                                        [128, 1], f32, tag="rcp", name=f"rce{t}{par}"
                                    )
                                    nc.vector.reciprocal(rc[:], pvq[par][:, t, 64:65])
                                    nc.vector.tensor_scalar_mul(
                                        stage[:, t, 64 * par : 64 * par + 64],
                                        pvq[par][:, t, 0:64],
                                        rc[:, 0:1],
                                    )
                                nc.sync.dma_start(
                                    att[:, hp, q0 + 128 * t : q0 + 128 * t + 128],
                                    stage[:, t, :],
                                    transpose=True,
                                )
                                for dseg in range(2):
                                    emit_outproj_piece(8 * half + 4 * s + t, dseg)
                    fill(FILL_ROWS)
                if eager_tail:
                    return
                # normalize + transpose into att; these free the PVQ banks
                # the next substream's PV chains are waiting on
                with tc.high_priority(offset=150):
                    for par in (0, 1):
                        rcp = RP.tile([128, 4], f32, tag="rcp4", name=f"rcp{half}{s}{hp}{par}")
                        nc.vector.reciprocal(rcp[:], pvq[par][:, :, 64])
                        for t in range(4):
                            nc.vector.tensor_scalar_mul(
                                stage[:, t, 64 * par : 64 * par + 64],
                                pvq[par][:, t, 0:64],
                                rcp[:, t : t + 1],
                            )
                    for t in range(4):
                        nc.sync.dma_start(
                            att[:, hp, q0 + 128 * t : q0 + 128 * t + 128],
                            stage[:, t, :],
                            transpose=True,
                        )

            # ---------- main emission ----------
            emit_qk_piece("q", 0, 0)
            emit_qk_piece("k", 0, 0)
            for sch in range(4):
                emit_v_piece(sch)
            xts[1] = emit_xt_load(1)
            nc.sync.dma_start(tri_sb[:], trid[:])
            nc.sync.dma_start(wo_sb[:], wo[:].rearrange("(c p) n -> p c n", p=128))
            push(("k", 0, 1), 3072, lambda: emit_qk_piece("k", 1, 0))
            push(("q", 0, 1), 3072, lambda: emit_qk_piece("q", 1, 0))
            push_qkv(1)

            SUBS = [(h, s, hp) for h, s in ((0, 0), (0, 1), (1, 0), (1, 1)) for hp in (0, 1)]
            for i, (half, s, hp) in enumerate(SUBS):
                if (half, s, hp) == (0, 0, 0):
                    xts[2] = emit_xt_load(2)
                    push_qkv(2)
                if (half, s, hp) == (0, 1, 0):
                    xts[3] = emit_xt_load(3)
                    push_qkv(3)
                # pull the next substream's qT piece ahead of this substream's
                # DVE backlog so its first scores aren't gated on the bias-add
                if i + 1 < len(SUBS):
                    nh, ns_, nhp = SUBS[i + 1]
                    nn4q = (1024 * nh + 512 * ns_) // 512
                    require(lambda k, n=nn4q, m=nhp: k[0] == "q" and k[1] == n and k[2] == m)
                emit_substream(half, s, hp, eager_tail=(i == len(SUBS) - 1))
                if hp == 1 and (half, s) != (1, 1):
                    for t in range(4):
                        T = 8 * half + 4 * s + t
                        for dseg in range(2):
                            push(
                                ("op", T, dseg),
                                1024,
                                lambda T=T, d=dseg: emit_outproj_piece(T, d),
                            )

            require(lambda k: True)

    nc.compile()
    return nc


_NC = None
_TRI = None


def _get_nc():
    global _NC, _TRI
    if _NC is None:
        _NC = build_nc()
        _TRI = make_tri()
    return _NC


def make_in_maps(x, Wq, bq, Wk, bk, Wv, bv, Wo):
    _get_nc()
    bf = ml_dtypes.bfloat16
    e4 = mybir.dt.np(fp8)
    x = np.asarray(x, np.float32)
    in_maps = []
    for core in range(8):
        b, g = core // 4, core % 4
        sl = slice(HG * g, HG * (g + 1))
        xt = np.ascontiguousarray(x[b].T)
        x8 = xt.astype(e4)
        x8r = (xt - x8.astype(np.float32)).astype(e4)
        m = {
            "x8": x8,
            "x8r": x8r,
            "bq": np.ascontiguousarray(np.asarray(bq, np.float32)[sl]),
            "bk": np.ascontiguousarray(np.asarray(bk, np.float32)[sl]),
            "bk8": np.ascontiguousarray(16.0 * np.asarray(bk, np.float32)[sl]).astype(e4),
            "bv": np.ascontiguousarray(WS * np.asarray(bv, np.float32)[sl]).astype(bf),
            "wo": np.ascontiguousarray(np.asarray(Wo, np.float32)[sl, :].astype(bf)),
            "tri": _TRI,
        }
        for nm, W in (("q", Wq), ("k", Wk), ("v", Wv)):
            ws = WS * np.asarray(W, np.float32)[:, sl]
            w8 = ws.astype(e4)
            w8r = (ws - w8.astype(np.float32)).astype(e4)
            # pre-rearranged (c p) m -> p c m so the DMA moves 2KB descriptors
            m[f"w8{nm}"] = np.ascontiguousarray(
                w8.reshape(8, 128, HG).transpose(1, 0, 2)
            )
            m[f"w8r{nm}"] = np.ascontiguousarray(
                w8r.reshape(8, 128, HG).transpose(1, 0, 2)
            )
        in_maps.append(m)
    return in_maps


def kernel(x, Wq, bq, Wk, bk, Wv, bv, Wo, _trace=False, _trace_kwargs=None):
    nc = _get_nc()
    in_maps = make_in_maps(x, Wq, bq, Wk, bk, Wv, bv, Wo)
    res = run_bass_kernel_spmd(
        nc, in_maps, list(range(8)), trace=_trace, **(_trace_kwargs or {})
    )
    out = np.zeros((2, N, D), np.float64)
    for core in range(8):
        out[core // 4] += np.asarray(res.results[core]["y"]).astype(np.float64)
    y = out.astype(np.float32)
    if _trace:
        return y, res
    return y
